# revision 34
# baseline (speedup 1.0000x reference)
"""Trainium2 Bass kernel for nn_DGN6 (gnn_message_passing).

Reference computation (per batch element, 3 rounds with K = 4, 8, 16):
    S = h @ h.T; causal top-K neighbors per row; msg = masked mean of
    neighbor rows; h = mom*h + (1-mom)*gelu((mix*h + (1-mix)*msg)*gain + bias)
Output: (h - x) * scale.

Distribution: data-parallel over B (2 batches), each batch's rows split
over 4 cores (8 cores total).  Core c handles batch c//4 and, within it,
4 row-blocks of 128 rows: blocks {cc + 4k, k=0..3} where cc = c%4 ("slot"
k holds block cc + 4k).  Every core runs an IDENTICAL instruction stream
(one SPMD program); per-core differences live entirely in input DATA
(causal masks, row data, per-row weights).

Numerics (all learned the hard way, measured on HW):
  * Everything stays fp32.  The reference's top-K selection sits on a
    cliff: quantizing h to bf16 (6e-2), f32r/tf32 (3.6e-2), or even
    bf16+fp8-residual (5e-2) flips near-tied selections and single flips
    cost ~0.05 absolute error -- over the 2e-2 gate.  fp32 measures 3e-7.
  * fp32 matmuls cost 4 PE cycles/row, but the timeline is bound by the
    AllGather pipeline (cost model: 15us + out_bytes/40GBps, serialized
    per core), so most of the PE cost hides under it.

Layout/dataflow per round, per slot:
  hT-build (rounds>0) PE-transposes hrows blocks into hT right before
  the score strips that need them; scores accumulate 512-wide strips
  (8 d-chunk matmuls each) into PSUM, masked-copied to SBUF with an
  additive causal mask (0/-3e38, per-strip streamed tiles); top-K via
  vector.max (+match_replace+max for K=16) -> threshold; M01 = (S>=th)
  bf16; M01 chunks PE-transposed 4-at-a-time through a rotating tile and
  immediately consumed by the aggregation matmuls (fp32, PSUM-accumulated
  over causal j-chunks); update u = (msg*w2 + h), h' = mom*h +
  (1-mom)*gelu(u*mix) with mix applied via the ACT engine's input scale
  (w2 = (1-mix)/(mix*cnt) per row); gain==1/bias==0 is detected on the
  host (the general affine path is compiled otherwise).  Round 3 folds
  momentum and (h-x)*scale into the output.

Inter-core exchange (the critical path): rows-only fp32 payload, per
boundary phases {0},{1},{2,3} then {0,1},{2},{3}, each AllGather fired
the moment its slots' updates are staged.  The phase schedule keeps the
collective pipeline 100%% busy from ~38us to ~547us (zero gaps), with
each round's compute overlapped under it:
  * score strips over already-arrived blocks are emitted before rebuilds
    of still-in-flight blocks (slot 3's early strips run during the
    phase waits);
  * back-DMAs (collective -> hrows) are deferred into the NEXT round's
    schedule at exactly the hT-build that consumes them: emitted any
    earlier they wait on their collective while blocking later traffic
    (mask loads, staging) on the same DMA queue;
  * scheduler fences (tc.no_sync_barrier) at round boundaries and before
    phase-gated rebuilds stop the static scheduler from hoisting
    collective-gated work ahead of ready work in the per-engine queues
    (priority inversions measured 30-90us each);
  * NOTHING computes on the Pool queue: collectives live there, and any
    instruction queued behind one waits out its full modeled duration.

All scalar parameters (sigmoid/softplus of the inputs) are applied on
the host into small input tensors, so the device program depends only
on shapes (and the gain/bias triviality flag).
"""

import math
import numpy as np

import concourse.bacc as bacc
import concourse.bass as bass
import concourse.mybir as mybir
import concourse.tile as tile
from concourse import bass_utils
from concourse.alu_op_type import AluOpType

F32 = mybir.dt.float32
F32R = mybir.dt.float32r
BF16 = mybir.dt.bfloat16
AF = mybir.ActivationFunctionType
BF16_NP = mybir.dt.np(BF16)

NEG_MASK = -3.0e38  # additive causal mask value (bf16-representable)
NEG_CLAMP = -1.0e29  # threshold clamp: above mask, below any real score

K_SCHEDULE = (4, 8, 16)


class Cfg:
    def __init__(self, B=2, T=2048, D=1024, G=4, S=4, bf16=False, affine=False):
        self.B, self.T, self.D, self.G, self.S = B, T, D, G, S
        self.bf16 = bf16
        self.affine = affine  # general gain/bias path
        self.P = 128
        self.DC = D // 128          # d-chunks
        self.NBLK = G * S           # row blocks per batch
        assert self.NBLK * 128 == T
        self.n_cores = B * G
        self.R = len(K_SCHEDULE)
        # slot k covers j-chunks [0, g(k)); block of core cc in slot k is cc + G*k
        self.g = [G * (k + 1) for k in range(S)]
        self.OFF = [128 * sum(self.g[:k]) for k in range(S)]  # mask free-dim offsets
        self.MTOT = 128 * sum(self.g)
        self.groups = [list(range(b * G, (b + 1) * G)) for b in range(B)]


def build_program(cfg: Cfg):
    """Build the single SPMD Bass/Tile program (identical on all cores)."""
    nc = bacc.Bacc(
        "TRN2", target_bir_lowering=False, debug=False,
        num_devices=cfg.n_cores,
    )
    P, D, T, DC, S, G, R = cfg.P, cfg.D, cfg.T, cfg.DC, cfg.S, cfg.G, cfg.R
    # CD: dtype of the shared h state and all matmul operands.  Plain fp32
    # (4 PE cycles/row) is required for exactness: f32r (1 cycle/row) rounds
    # operands to ~10-bit mantissa on real HW, which flips top-k selections
    # vs the fp32 reference (measured 3.6e-2 rel, over the 2e-2 gate); bf16
    # is worse still (6e-2).  The score/agg matmuls are hidden under the
    # AllGather chain, so the 4x PE cost is mostly free.
    CD = BF16 if cfg.bf16 else F32
    TD = BF16 if cfg.bf16 else F32

    def mmcast(ap):
        return ap

    # ---- I/O ----
    i_hT = nc.dram_tensor("i_hT", [P, DC * T], CD, kind="ExternalInput")
    i_hr = nc.dram_tensor("i_hr", [P, cfg.NBLK * D], CD, kind="ExternalInput")
    i_myh = nc.dram_tensor("i_myh", [P, S * D], F32, kind="ExternalInput")
    i_hTm = nc.dram_tensor("i_hTm", [P, S * D], CD, kind="ExternalInput")
    i_msk = nc.dram_tensor("i_msk", [P, cfg.MTOT], BF16, kind="ExternalInput")
    i_xs = nc.dram_tensor("i_xs", [P, S * D], F32, kind="ExternalInput")
    i_w2 = nc.dram_tensor("i_w2", [P, R * S], F32, kind="ExternalInput")
    # per-partition scalar params: col 0 = mom, 1 = s*(1-mom), 2 = s*mom,
    # 3 = 1-mom, 4+r = mix_r (gelu input scale on the trivial-affine path)
    i_sc = nc.dram_tensor("i_sc", [P, 8], F32, kind="ExternalInput")
    i_idc = nc.dram_tensor("i_idc", [P, 128], CD, kind="ExternalInput")
    i_idf = nc.dram_tensor("i_idf", [P, 128], F32, kind="ExternalInput")
    i_idb = nc.dram_tensor("i_idb", [P, 128], BF16, kind="ExternalInput")
    if cfg.affine:
        i_gm = nc.dram_tensor("i_gm", [R, P, D], F32, kind="ExternalInput")
        i_bb = nc.dram_tensor("i_bb", [R, P, D], F32, kind="ExternalInput")
    o_out = nc.dram_tensor("o_out", [S, P, D], F32, kind="ExternalOutput")

    NH = D // 512  # 512-wide halves of D
    with tile.TileContext(nc) as tc:
        with (
            tc.tile_pool(name="const", bufs=1) as const,
            tc.tile_pool(name="work", bufs=2) as work,
            tc.tile_pool(name="psum", bufs=2, space="PSUM") as psum,
            tc.tile_pool(name="dram", bufs=1, space="DRAM") as dram,
        ):
            # ---- persistent state ----
            hT = const.tile([P, DC * T], CD, name="hT")
            hrows = const.tile([P, cfg.NBLK * D], CD, name="hrows")
            myh = const.tile([P, S * D], F32, name="myh")
            hTm = const.tile([P, S * D], CD, name="hTm")
            w2t = const.tile([P, R * S], F32, name="w2t")
            sct = const.tile([P, 8], F32, name="sct")
            idc = const.tile([P, 128], CD, name="idc")
            idf = const.tile([P, 128], F32, name="idf")
            idb = const.tile([P, 128], BF16, name="idb")

            hTv = hT.rearrange("p (c j) -> p c j", c=DC)
            iTv = i_hT[:].rearrange("p (c j) -> p c j", c=DC)

            # Initial loads in strict first-use order, ONLY what slot-0's
            # chain needs: its staging DMA gates the first phase collective
            # and every later phase chains off it (the collective pipeline
            # is gap-free), so every early microsecond here moves the whole
            # timeline.  All DMA transfers serialize on the one modeled
            # DMA-engines device, so non-critical bytes (rest of hTm/myh,
            # later stripes) are deferred below into the round-0 schedule.
            j1_0 = cfg.g[0]
            nc.sync.dma_start(hTm[:, 0:D // 2], i_hTm[:, 0:D // 2])
            nc.sync.dma_start(hTm[:, D // 2:D], i_hTm[:, D // 2:D])
            # slot-0's first score matmul needs only chunk dc=0 of the hT
            # window; per-dc loads let the PE start ~7us earlier.
            for dc in range(DC):
                hw_ = j1_0 * 128 // 2
                for hf in range(2):
                    nc.sync.dma_start(hTv[:, dc:dc + 1, hf * hw_:(hf + 1) * hw_],
                                      iTv[:, dc:dc + 1, hf * hw_:(hf + 1) * hw_])
            nc.sync.dma_start(hrows[:, 0:j1_0 * D], i_hr[:, 0:j1_0 * D])
            nc.sync.dma_start(idb[:], i_idb[:])  # gates slot-0's m01 transpose
            nc.sync.dma_start(myh[:, 0:D], i_myh[:, 0:D])
            nc.sync.dma_start(sct[:], i_sc[:])
            w2dma = nc.sync.dma_start(w2t[:], i_w2[:])
            nc.sync.dma_start(idc[:], i_idc[:])
            nc.sync.dma_start(idf[:], i_idf[:])
            # Warm the ACT Gelu table at t~0: the 1.28us table load otherwise
            # lands on slot-0's critical path, which gates the first phase
            # collective and with it the whole gap-free collective chain.
            warm = const.tile([P, 1], F32, name="warm")
            nc.vector.memset(warm[:], 0.0)
            nc.scalar.activation(warm[:], warm[:], AF.Gelu)

            def rest_loads():
                nc.sync.dma_start(hTm[:, D:S * D], i_hTm[:, D:S * D])
                nc.sync.dma_start(myh[:, D:S * D], i_myh[:, D:S * D])

            # Bulk hT/hrows loads are NOT on the Pool queue: Pool DMAs hold
            # the Pool engine for the whole transfer in the cost model, and
            # the first phase collective (also on Pool) would queue behind
            # all 38us of them, delaying the whole (chain-bound) collective
            # pipeline.  They ride the scalar HWDGE queue instead.  The
            # first one is gated on the last small critical load: all DMA
            # transfers serialize on the one modeled DMA-engines device and
            # Tile otherwise hoists these dep-free issues to t~0, starving
            # slot-0's staging (which gates the whole collective chain).
            from concourse.tile import add_dep_helper as _add_dep

            def bulk_load(k, gate=None):
                j0, j1 = cfg.g[k - 1], cfg.g[k]
                d1 = nc.scalar.dma_start(hTv[:, :, j0 * 128:j1 * 128],
                                         iTv[:, :, j0 * 128:j1 * 128])
                d2 = nc.scalar.dma_start(hrows[:, j0 * D:j1 * D],
                                         i_hr[:, j0 * D:j1 * D])
                if gate is not None:
                    for d_ in (d1, d2):
                        _add_dep(d_.ins, gate.ins, sync=True,
                                 reason="bulk loads start after slot-0 critical loads")

            ap_mom = sct[:, 0:1]
            ap_s1m = sct[:, 1:2]
            ap_sm = sct[:, 2:3]
            ap_1m = sct[:, 3:4]

            # per-round, per-phase AllGather buffers (DRAM), rows-only payload.
            # Boundary 0 ships slots {0,1} then {2,3}; boundary 1 (feeding the
            # final round) splits the tail phases so the last round's slot-2/3
            # chains start as soon as their own blocks arrive.
            PHS = [[[0], [1], [2, 3]], [[0, 1], [2], [3]]][:R - 1]
            ag_in = [[dram.tile([len(ph), P, D], CD, name=f"ag_in{r}_{p}", tag=f"agi{r}_{p}")
                      for p, ph in enumerate(PHS[r])] for r in range(R - 1)]
            ag_out = [[dram.tile([len(ph) * G, P, D], CD, name=f"ag_out{r}_{p}", tag=f"ago{r}_{p}")
                       for p, ph in enumerate(PHS[r])] for r in range(R - 1)]
            # slot -> (phase, q) per boundary
            PQ = [{k: (p, q) for p, ph in enumerate(phs) for q, k in enumerate(ph)}
                  for phs in PHS]

            # alternate psum->sbuf copy engines to balance DVE/ACT load
            _alt = [0]
            pending_backs = [{}]  # slot -> deferred back-DMA emitter

            def copy_out(dst, src):
                _alt[0] ^= 1
                if _alt[0]:
                    nc.vector.tensor_copy(dst, src)
                else:
                    nc.scalar.activation(dst, src, AF.Copy)

            for r in range(R):
                K = K_SCHEDULE[r]

                # propagation of slot k-1 is deferred until slot k's scores are
                # queued (the h'-transposes wait on the DVE/ACT elementwise
                # chain and would otherwise stall the next slot's matmuls on
                # the strict PE FIFO).
                pending_prop = None

                def build_blocks(b0, b1, rr=r):
                    # PE-transpose hrows blocks [b0,b1) into hT columns
                    for blk in range(b0, b1):
                        for half in range(2):
                            ptr = psum.tile([P, 512], CD, tag="pt", bufs=2,
                                            name=f"ptr_{rr}_{blk}_{half}")
                            for i4 in range(4):
                                dc = half * 4 + i4
                                nc.tensor.transpose(
                                    ptr[:, i4 * 128:(i4 + 1) * 128],
                                    hrows[:, blk * D + dc * 128: blk * D + (dc + 1) * 128],
                                    idc[:])
                            dst = hTv[:, half * 4:half * 4 + 4,
                                      blk * 128:(blk + 1) * 128]
                            src = ptr[:].rearrange("p (c j) -> p c j", c=4)
                            copy_out(dst, src)

                scs = {}

                def ensure_sc(kk, rr=r):
                    if kk not in scs:
                        W = cfg.g[kk] * 128
                        scs[kk] = work.tile([P, W], F32, tag="sc", bufs=2,
                                            name=f"sc_{rr}_{kk}")

                def score_strips(kk, w0_list, rr=r):
                    ensure_sc(kk)
                    for w0 in w0_list:
                        # per-strip causal mask chunk (small rotating bufs)
                        mskt = work.tile([P, 512], BF16, tag="msk",
                                         bufs=3 if not cfg.affine else 2,
                                         name=f"msk_{rr}_{kk}_{w0}")
                        nc.scalar.dma_start(
                            mskt[:], i_msk[:, cfg.OFF[kk] + w0:cfg.OFF[kk] + w0 + 512])
                        ps = psum.tile([P, 512], F32, tag="ps_sc", bufs=2,
                                       name=f"ps_{rr}_{kk}_{w0}")
                        for dc in range(DC):
                            nc.tensor.matmul(
                                ps[:],
                                mmcast(hTm[:, (kk * DC + dc) * 128:(kk * DC + dc + 1) * 128]),
                                mmcast(hT[:, dc * T + w0: dc * T + w0 + 512]),
                                start=(dc == 0), stop=(dc == DC - 1),
                            )
                        # masked copy PSUM -> SBUF: sc = S + mask (0 / -3e38)
                        nc.vector.scalar_tensor_tensor(
                            scs[kk][:, w0:w0 + 512], ps[:], 1.0, mskt[:],
                            AluOpType.mult, AluOpType.add,
                        )

                def seg16_into(dst16, ap, W, tagsfx, rr=r):
                    """Top-16 candidates of `ap` ([P, W<=512]) into dst16
                    ([P, 16]): top-8, then ranks 9-16 via match_replace.  The
                    scratch rides the shared bufs=1 [P, 512] "scr" tag
                    (sequential reuses serialize via WAR)."""
                    nc.vector.max(out=dst16[:, 0:8], in_=ap)
                    scr = work.tile([P, W], F32, tag="scr",
                                    bufs=1, name=f"scr_{rr}_{tagsfx}")
                    nc.vector.match_replace(out=scr[:], in_to_replace=dst16[:, 0:8],
                                            in_values=ap, imm_value=NEG_MASK)
                    nc.vector.max(out=dst16[:, 8:16], in_=scr[:])

                def top16_th(sc_ap, W, tagsfx, rr=r):
                    """16th-largest source AP over a [P, W] score range,
                    per-512-segment candidates then a merge pass — a flat
                    match_replace over W=2048 would need an 8KB/partition
                    scratch; segments cap it at 2KB."""
                    nseg = (W + 511) // 512
                    candc = work.tile([P, nseg * 16], F32, tag="candc",
                                      bufs=1, name=f"candc_{rr}_{tagsfx}")
                    for s_ in range(nseg):
                        lo = s_ * 512
                        seg16_into(candc[:, s_ * 16:(s_ + 1) * 16],
                                   sc_ap[:, lo:min(lo + 512, W)],
                                   min(512, W - lo), f"{tagsfx}{s_}")
                    if nseg == 1:
                        return candc[:, 8:16]
                    mg = work.tile([P, 16], F32, tag="mg", bufs=1,
                                   name=f"mg_{rr}_{tagsfx}")
                    seg16_into(mg[:], candc[:], nseg * 16, f"{tagsfx}m")
                    return mg[:, 8:16]

                hb_pre = {}
                _hbalt = [0]

                def hb_copy(jc, h_, rr=r):
                    """fp32 hrows chunk -> rotating bf16 tile (the rounding
                    producer the BIR verifier wants for bf16 matmuls).
                    DVE-heavy 2:1 engine split: the modeled DVE copy is
                    0.39us vs ACT's 0.61us for [128, 512]."""
                    hb = work.tile([P, 512], BF16, tag="hb", bufs=6,
                                   name=f"hb_{rr}_{jc}_{h_}")
                    hop = hrows[:, jc * D + h_ * 512: jc * D + h_ * 512 + 512]
                    _hbalt[0] = (_hbalt[0] + 1) % 3
                    if _hbalt[0] == 0:
                        nc.scalar.activation(hb[:], hop, AF.Copy)
                    else:
                        nc.vector.tensor_copy(hb[:], hop)
                    return hb

                def slot_rest(k, rr=r, th_src_in=None, fast_agg=False):
                    """topk -> mask -> aggregate -> update -> defer prop."""
                    nonlocal pending_prop
                    g = cfg.g[k]
                    W = g * 128
                    sc = scs[k]
                    # ---- top-K threshold ----
                    if th_src_in is not None:
                        th_src = th_src_in
                    elif K <= 8:
                        mx = work.tile([P, 8], F32, tag="mx", name=f"mx_{rr}_{k}")
                        nc.vector.max(out=mx[:], in_=sc[:])
                        th_src = mx[:, K - 1:K]
                    else:
                        t16 = top16_th(sc, W, f"f{k}")
                        th_src = t16[:, K - 9:K - 8]
                    th = work.tile([P, 1], F32, tag="th", name=f"th_{rr}_{k}")
                    nc.vector.tensor_scalar_max(th[:], th_src, NEG_CLAMP)
                    # ---- 0/1 neighbor mask.  MUST NOT ride the Pool queue:
                    # collectives live there, and any instruction queued
                    # behind one waits for its (120us) completion. ----
                    m01 = work.tile([P, W], BF16, tag="m01", bufs=1, name=f"m01_{rr}_{k}")
                    nc.vector.tensor_scalar(m01[:], sc[:], th[:], None, AluOpType.is_ge)
                    # ---- transpose mask chunks + aggregate (interleaved):
                    # msg_raw = M01 @ hrows, accumulated 4 j-chunks at a time
                    # through a small rotating transposed-mask tile ----
                    pss = [psum.tile([P, 512], F32, tag="ps_ag", bufs=2,
                                     name=f"pa_{rr}_{k}_{h_}") for h_ in range(NH)]
                    for jq in range(0, g, 4):
                        ptm = psum.tile([P, 512], BF16, tag="ptb", bufs=2,
                                        name=f"ptm_{rr}_{k}_{jq}")
                        for i4 in range(4):
                            nc.tensor.transpose(
                                ptm[:, i4 * 128:(i4 + 1) * 128],
                                m01[:, (jq + i4) * 128:(jq + i4 + 1) * 128], idb[:])
                        # fast_agg (last round, tail slot only): bf16
                        # aggregation at 1 PE cycle/row instead of fp32's 4.
                        # The mask operand is 0/1 (exact); the h operand is
                        # rounded to bf16 by ACT/DVE chunk copies (the BIR
                        # verifier requires low-precision matmul operands to
                        # come from rounding producers, so no bitcast).
                        # Aggregation error here reaches only the OUTPUT
                        # (~1e-4 rel, no further top-k rounds to flip);
                        # earlier rounds stay fp32 — their msg error would
                        # perturb the next round's scores ~45x and flip
                        # selections.
                        mdt = BF16 if fast_agg else CD
                        mts = work.tile([P, 512], mdt, tag="mt",
                                        bufs=2 if not cfg.affine else 1,
                                        name=f"mt_{rr}_{k}_{jq}")
                        copy_out(mts[:], ptm[:])
                        for h_ in range(NH):
                            for i4 in range(4):
                                jc = jq + i4
                                hop = hrows[:, jc * D + h_ * 512: jc * D + h_ * 512 + 512]
                                if fast_agg:
                                    hb = hb_pre.pop((jc, h_), None)
                                    if hb is None:
                                        hb = hb_copy(jc, h_)
                                    hop = hb[:]
                                nc.tensor.matmul(
                                    pss[h_][:], mmcast(mts[:, i4 * 128:(i4 + 1) * 128]),
                                    mmcast(hop),
                                    start=(jc == 0), stop=(jc == g - 1))
                    # ---- elementwise update ----
                    w2ap = w2t[:, rr * S + k: rr * S + k + 1]
                    for h_ in range(NH):
                        sl = slice(k * D + h_ * 512, k * D + (h_ + 1) * 512)
                        hsl = slice(h_ * 512, (h_ + 1) * 512)
                        t1 = work.tile([P, 512], F32, tag="t1",
                                       bufs=2 if not cfg.affine else 1,
                                       name=f"t1_{rr}_{k}_{h_}")
                        nc.vector.scalar_tensor_tensor(
                            t1[:], pss[h_][:], w2ap, myh[:, sl],
                            AluOpType.mult, AluOpType.add)
                        if cfg.affine:
                            gmt = work.tile([P, 512], F32, tag="gmt", bufs=2,
                                            name=f"gmt_{rr}_{k}_{h_}")
                            nc.scalar.dma_start(gmt[:], i_gm[rr][:, hsl])
                            bbt = work.tile([P, 512], F32, tag="bbt", bufs=2,
                                            name=f"bbt_{rr}_{k}_{h_}")
                            nc.scalar.dma_start(bbt[:], i_bb[rr][:, hsl])
                            nc.vector.tensor_mul(t1[:], t1[:], gmt[:])
                            nc.vector.tensor_add(t1[:], t1[:], bbt[:])
                        gl = work.tile([P, 512], F32, tag="gl",
                                       bufs=2 if not cfg.affine else 1,
                                       name=f"gl_{rr}_{k}_{h_}")
                        if cfg.affine:
                            nc.scalar.activation(gl[:], t1[:], AF.Gelu)
                        else:
                            # u = (psum*w2 + h) * mix  (w2 carries the /mix)
                            nc.scalar.activation(gl[:], t1[:], AF.Gelu,
                                                 scale=sct[:, 4 + rr:5 + rr])
                        if rr < R - 1:
                            nc.vector.tensor_scalar_mul(gl[:], gl[:], ap_1m)
                            # in-place: myh = mom*myh + (1-mom)*gelu
                            nc.vector.scalar_tensor_tensor(
                                myh[:, sl], myh[:, sl], ap_mom, gl[:],
                                AluOpType.mult, AluOpType.add)
                        else:
                            xst = work.tile([P, 512], F32, tag="xst", bufs=1,
                                            name=f"xst_{rr}_{k}_{h_}")
                            nc.sync.dma_start(xst[:], i_xs[:, sl])
                            # gl <- s*(1-mom)*gelu - s*x
                            nc.vector.scalar_tensor_tensor(
                                gl[:], gl[:], ap_s1m, xst[:],
                                AluOpType.mult, AluOpType.subtract)
                            # t1 <- s*mom*h + gl
                            nc.vector.scalar_tensor_tensor(
                                t1[:], myh[:, sl], ap_sm, gl[:],
                                AluOpType.mult, AluOpType.add)
                            nc.sync.dma_start(o_out[k, :, hsl], t1[:])
                    # ---- propagate h' (transposed into hTm + staged rows) ----
                    if rr < R - 1:
                        def _prop(kk=k):
                            if cfg.bf16:
                                hi_t = work.tile([P, D], CD, tag="hi_t", bufs=1,
                                                 name=f"hi_{rr}_{kk}")
                                nc.scalar.activation(
                                    hi_t[:], myh[:, kk * D:(kk + 1) * D], AF.Copy)
                                src = hi_t[:]
                            else:
                                src = myh[:, kk * D:(kk + 1) * D]
                            for half in range(2):
                                pto = psum.tile([P, 512], TD, tag="pt", bufs=2,
                                                name=f"pto_{rr}_{kk}_{half}")
                                for i4 in range(4):
                                    dc = half * 4 + i4
                                    nc.tensor.transpose(
                                        pto[:, i4 * 128:(i4 + 1) * 128],
                                        src[:, dc * 128:(dc + 1) * 128],
                                        idc[:] if cfg.bf16 else idf[:])
                                copy_out(hTm[:, (kk * DC + half * 4) * 128:
                                              (kk * DC + half * 4 + 4) * 128], pto[:])
                            p_, q_ = PQ[rr][kk]
                            nc.sync.dma_start(ag_in[rr][p_][q_], src)
                            # fire the phase collective as soon as all its
                            # slots are staged; it only touches DRAM, so it
                            # overlaps the remaining slots' compute.
                            if q_ == len(PHS[rr][p_]) - 1 and p_ < len(PHS[rr]) - 1:
                                nc.gpsimd.collective_compute(
                                    "AllGather", AluOpType.bypass,
                                    replica_groups=cfg.groups,
                                    ins=[ag_in[rr][p_].opt()],
                                    outs=[ag_out[rr][p_].opt()])
                        pending_prop = _prop

                def fire_prop():
                    nonlocal pending_prop
                    if pending_prop is not None:
                        with tc.high_priority():
                            pending_prop()
                        pending_prop = None

                # ---- the round's emission schedule.  The PE FIFO is strict,
                # so emission order = execution order: score strips over
                # already-built hT blocks are emitted before rebuilds whose
                # back-DMA may still be waiting on a phase collective, and
                # slot 3's early strips run during the phase-1 wait.
                if r == 0:
                    for k in range(S):
                        score_strips(k, range(0, cfg.g[k] * 128, 512))
                        fire_prop()
                        slot_rest(k)
                        if k == 0:
                            rest_loads()
                        if k < S - 1:
                            # hT/hrows for the NEXT slot
                            bulk_load(k + 1, gate=w2dma)
                else:
                    # scheduler fence at the round boundary: without it the
                    # static scheduler interleaves this round's DVE/ACT ops
                    # before the previous round's tail updates in the per-
                    # engine queues, stalling the staging of the boundary
                    # collectives behind collective-gated work.
                    tc.no_sync_barrier()

                    def fire_backs(at_slot):
                        em = pending_backs[0].pop(at_slot, None)
                        if em is not None:
                            em()

                    fire_backs(0)
                    build_blocks(0, G)
                    score_strips(0, [0])
                    slot_rest(0)
                    score_strips(1, [0])
                    fire_backs(1)
                    build_blocks(G, 2 * G)
                    score_strips(1, [512])
                    fire_prop()  # prop(0): stages the next boundary's phase 0
                    slot_rest(1)
                    score_strips(2, [0, 512])
                    fire_prop()  # prop(1) EARLY: its staging must not queue
                    #              behind the phase-dependent strips below
                    score_strips(3, [0, 512])
                    tc.no_sync_barrier()  # keep the phase-gated rebuild below
                    #                       from being scheduled before the
                    #                       ready work above
                    fire_backs(2)
                    build_blocks(2 * G, 3 * G)
                    score_strips(2, [1024])
                    score_strips(3, [1024])
                    slot_rest(2)
                    fire_prop()  # prop(2) early, same reason
                    if r == R - 1:
                        # Prefix top-16 candidates over the already-computed
                        # [0, 1536) scores, emitted AFTER slot-2's DVE chain
                        # so it doesn't push slot-2's (zero-slack) work into
                        # the tail.  Runs hidden under the last collective.
                        candt = work.tile([P, 64], F32, tag="candc", bufs=1,
                                          name=f"candt_{r}_3")
                        for s_ in range(3):
                            seg16_into(candt[:, s_ * 16:(s_ + 1) * 16],
                                       scs[3][:, s_ * 512:(s_ + 1) * 512],
                                       512, f"p3{s_}")
                    tc.no_sync_barrier()
                    fire_backs(3)
                    if r < R - 1:
                        build_blocks(3 * G, 4 * G)
                        score_strips(3, [1536])
                        slot_rest(3)
                    else:
                        # ---- last-round tail: everything after the final
                        # collective is serial program end, so the stripe-3
                        # work is pipelined per BLOCK (each block's hT
                        # rebuild + 128-wide score segment fires as soon as
                        # its own rank's back-DMA lands) and the top-16
                        # threshold merges the hidden prefix candidates with
                        # candidates from the just-arrived 512-wide suffix
                        # instead of re-scanning all 2048 columns. ----
                        ensure_sc(3)
                        w0 = 1536
                        mskt = work.tile([P, 512], BF16, tag="msk", bufs=3,
                                         name=f"msk_{r}_3_tail")
                        nc.scalar.dma_start(
                            mskt[:], i_msk[:, cfg.OFF[3] + w0:cfg.OFF[3] + w0 + 512])
                        ps = psum.tile([P, 512], F32, tag="ps_sc", bufs=2,
                                       name=f"ps_{r}_3_tail")
                        # builds and strip segments interleaved one build
                        # ahead: each build's psum->SBUF copy costs a DVE
                        # round-trip, which the PREVIOUS block's strip
                        # matmuls hide.
                        def strip_seg(q):
                            for dc in range(DC):
                                nc.tensor.matmul(
                                    ps[:, q * 128:(q + 1) * 128],
                                    mmcast(hTm[:, (3 * DC + dc) * 128:(3 * DC + dc + 1) * 128]),
                                    mmcast(hT[:, dc * T + w0 + q * 128:
                                              dc * T + w0 + (q + 1) * 128]),
                                    start=(dc == 0), stop=(dc == DC - 1),
                                )
                        build_blocks(3 * G, 3 * G + 1)
                        build_blocks(3 * G + 1, 3 * G + 2)
                        strip_seg(0)
                        build_blocks(3 * G + 2, 3 * G + 3)
                        strip_seg(1)
                        build_blocks(3 * G + 3, 3 * G + 4)
                        strip_seg(2)
                        strip_seg(3)
                        nc.vector.scalar_tensor_tensor(
                            scs[3][:, w0:w0 + 512], ps[:], 1.0, mskt[:],
                            AluOpType.mult, AluOpType.add,
                        )
                        for (jc_, h2_) in [(0, 0), (1, 0), (2, 0),
                                           (3, 0), (0, 1), (1, 1)]:
                            hb_pre[(jc_, h2_)] = hb_copy(jc_, h2_)
                        seg16_into(candt[:, 48:64], scs[3][:, w0:w0 + 512],
                                   512, "s3")
                        mgt = work.tile([P, 16], F32, tag="mg", bufs=1,
                                        name=f"mgt_{r}_3")
                        seg16_into(mgt[:], candt[:], 64, "m3")
                        slot_rest(3, th_src_in=mgt[:, 15:16], fast_agg=True)

                fire_prop()  # prop(3)

                # ---- round boundary: last phase collective + back-DMAs ----
                # Back-DMAs are emitted after the slot loop so Tile orders
                # them after this round's readers of hrows.  The fence keeps
                # the scheduler from hoisting them ahead of this round's tail
                # ops on the shared DMA queues (priority inversion: a back-DMA
                # WAR-waits on this round's aggregation, which itself needs a
                # psum copy queued behind that same back-DMA).
                if r < R - 1:
                    tc.no_sync_barrier()
                    nph = len(PHS[r])
                    nc.gpsimd.collective_compute(
                        "AllGather", AluOpType.bypass, replica_groups=cfg.groups,
                        ins=[ag_in[r][nph - 1].opt()], outs=[ag_out[r][nph - 1].opt()])

                    def _backs(p_, eng, rr=r):
                        # one DMA per (phase, q, rank): finer pieces let the
                        # tail's per-block hT rebuilds start as soon as THEIR
                        # rank's rows land instead of waiting out one big
                        # strided transfer on the serial DMA device.
                        nq = len(PHS[rr][p_])
                        srcv = ag_out[rr][p_][:].rearrange(
                            "(rnk q) p d -> q rnk p d", q=nq)
                        for q in range(nq):
                            base = PHS[rr][p_][q] * G
                            for rnk in range(G):
                                for hf in range(2):
                                    eng.dma_start(
                                        hrows[:, (base + rnk) * D + hf * (D // 2):
                                               (base + rnk) * D + (hf + 1) * (D // 2)],
                                        srcv[q, rnk, :, hf * (D // 2):(hf + 1) * (D // 2)])

                    # ALL back-DMAs are deferred into the next round's
                    # schedule (sync queue), each emitted just before the
                    # hT rebuild that consumes it: emitted any earlier they
                    # sit on a DMA queue WAITING on their phase collective,
                    # blocking later traffic on that queue (mask loads,
                    # staging) that the next round needs much sooner.
                    pending_backs[0] = {
                        PHS[r][p_][0]: (lambda rr=r, pp=p_: _backs(pp, nc.sync, rr))
                        for p_ in range(nph)}

    nc.compile()
    return nc


# ------------------------------------------------------------------
# Host side
# ------------------------------------------------------------------

def _sigmoid(v):
    return 1.0 / (1.0 + math.exp(-float(v)))


def prep_inputs(cfg: Cfg, x, gain, bias, log_mix, log_momentum, log_scale):
    """Build the per-core input maps (numpy)."""
    P, D, T, DC, S, G, R = cfg.P, cfg.D, cfg.T, cfg.DC, cfg.S, cfg.G, cfg.R
    cd_np = BF16_NP if cfg.bf16 else np.float32
    x = np.asarray(x, np.float32)
    gain = np.asarray(gain, np.float32)
    bias = np.asarray(bias, np.float32)
    mix = np.array([_sigmoid(v) for v in np.asarray(log_mix, np.float32)], np.float64)
    mom = _sigmoid(log_momentum)
    s = math.log1p(math.exp(float(log_scale))) + 0.01

    scl = np.zeros((P, 8), np.float32)
    scl[:, 0] = mom
    scl[:, 1] = s * (1.0 - mom)
    scl[:, 2] = s * mom
    scl[:, 3] = 1.0 - mom
    scl[:, 4:4 + R] = mix.astype(np.float32)[None, :]
    idc = np.eye(128, dtype=cd_np)

    common = {"i_sc": scl, "i_idc": idc, "i_idb": np.eye(128, dtype=BF16_NP),
              "i_idf": np.eye(128, dtype=np.float32)}
    if cfg.affine:
        common["i_gm"] = np.ascontiguousarray(np.broadcast_to(
            (gain * mix[:, None].astype(np.float32)).astype(np.float32)[:, None, :],
            (R, P, D)))
        common["i_bb"] = np.ascontiguousarray(np.broadcast_to(bias[:, None, :], (R, P, D)))

    in_maps = []
    for c in range(cfg.n_cores):
        b, cc = divmod(c, G)
        blocks = [cc + G * k for k in range(S)]
        h0 = x[b]  # [T, D]
        hT0 = np.ascontiguousarray(
            h0.T.reshape(DC, 128, T).transpose(1, 0, 2)).reshape(128, DC * T).astype(cd_np)
        hr0 = np.ascontiguousarray(
            h0.astype(cd_np).reshape(cfg.NBLK, 128, D).transpose(1, 0, 2)).reshape(128, cfg.NBLK * D)
        hb = h0.reshape(cfg.NBLK, 128, D)[blocks]  # [S,128,D]
        myh0 = np.ascontiguousarray(hb.transpose(1, 0, 2)).reshape(128, S * D)
        hTm0 = np.ascontiguousarray(
            hb.reshape(S, 128, DC, 128).transpose(3, 0, 2, 1)).reshape(128, S * D).astype(cd_np)
        msk = np.zeros((P, cfg.MTOT), BF16_NP)
        for k in range(S):
            W = cfg.g[k] * 128
            rowid = blocks[k] * 128 + np.arange(128)  # [128]
            j = np.arange(W)
            m = np.where(j[None, :] <= rowid[:, None], 0.0, NEG_MASK).astype(BF16_NP)
            msk[:, cfg.OFF[k]:cfg.OFF[k] + W] = m
        w2 = np.zeros((P, R * S), np.float32)
        for r in range(R):
            for k in range(S):
                cnt = np.minimum(blocks[k] * 128 + np.arange(128) + 1, K_SCHEDULE[r])
                w2[:, r * S + k] = ((1.0 - mix[r]) / (mix[r] * cnt)).astype(np.float32)
        in_maps.append(dict(common,
                            i_hT=hT0, i_hr=hr0, i_myh=myh0, i_hTm=hTm0,
                            i_msk=msk, i_xs=(s * myh0.astype(np.float64)).astype(np.float32),
                            i_w2=w2))
    return in_maps


def assemble_output(cfg: Cfg, results, dtype=np.float32):
    """results: list (per core) of {'o_out': [S,128,D]} -> full [B,T,D]."""
    out = np.zeros((cfg.B, cfg.T, cfg.D), dtype)
    for c in range(cfg.n_cores):
        b, cc = divmod(c, cfg.G)
        o = results[c]["o_out"]
        for k in range(cfg.S):
            blk = cc + cfg.G * k
            out[b, blk * 128:(blk + 1) * 128] = o[k]
    return out


_PROGRAM_CACHE = {}


def _get_program(cfg: Cfg):
    key = (cfg.B, cfg.T, cfg.D, cfg.G, cfg.S, cfg.bf16, cfg.affine)
    if key not in _PROGRAM_CACHE:
        _PROGRAM_CACHE[key] = build_program(cfg)
    return _PROGRAM_CACHE[key]


def run(cfg: Cfg, inputs: dict, trace: bool = False):
    nc = _get_program(cfg)
    in_maps = prep_inputs(cfg, **inputs)
    res = bass_utils.run_bass_kernel_spmd(
        nc, in_maps, list(range(cfg.n_cores)), trace=trace)
    out = assemble_output(cfg, res.results)
    return out, res


def kernel(**inputs) -> np.ndarray:
    trivial = (np.all(np.asarray(inputs["gain"]) == 1.0)
               and np.all(np.asarray(inputs["bias"]) == 0.0))
    cfg = Cfg(affine=not trivial)  # B=2, T=2048, D=1024, 8 cores
    out, _ = run(cfg, inputs)
    return out.astype(np.float32)



# revision 35
# speedup vs baseline: 1.0142x; 1.0142x over previous
"""Trainium2 Bass kernel for nn_DGN6 (gnn_message_passing).

Reference computation (per batch element, 3 rounds with K = 4, 8, 16):
    S = h @ h.T; causal top-K neighbors per row; msg = masked mean of
    neighbor rows; h = mom*h + (1-mom)*gelu((mix*h + (1-mix)*msg)*gain + bias)
Output: (h - x) * scale.

Distribution: data-parallel over B (2 batches), each batch's rows split
over 4 cores (8 cores total).  Core c handles batch c//4 and, within it,
4 row-blocks of 128 rows: blocks {cc + 4k, k=0..3} where cc = c%4 ("slot"
k holds block cc + 4k).  Every core runs an IDENTICAL instruction stream
(one SPMD program); per-core differences live entirely in input DATA
(causal masks, row data, per-row weights).

Numerics (all learned the hard way, measured on HW):
  * Everything stays fp32.  The reference's top-K selection sits on a
    cliff: quantizing h to bf16 (6e-2), f32r/tf32 (3.6e-2), or even
    bf16+fp8-residual (5e-2) flips near-tied selections and single flips
    cost ~0.05 absolute error -- over the 2e-2 gate.  fp32 measures 3e-7.
  * fp32 matmuls cost 4 PE cycles/row, but the timeline is bound by the
    AllGather pipeline (cost model: 15us + out_bytes/40GBps, serialized
    per core), so most of the PE cost hides under it.

Layout/dataflow per round, per slot:
  hT-build (rounds>0) PE-transposes hrows blocks into hT right before
  the score strips that need them; scores accumulate 512-wide strips
  (8 d-chunk matmuls each) into PSUM, masked-copied to SBUF with an
  additive causal mask (0/-3e38, per-strip streamed tiles); top-K via
  vector.max (+match_replace+max for K=16) -> threshold; M01 = (S>=th)
  bf16; M01 chunks PE-transposed 4-at-a-time through a rotating tile and
  immediately consumed by the aggregation matmuls (fp32, PSUM-accumulated
  over causal j-chunks); update u = (msg*w2 + h), h' = mom*h +
  (1-mom)*gelu(u*mix) with mix applied via the ACT engine's input scale
  (w2 = (1-mix)/(mix*cnt) per row); gain==1/bias==0 is detected on the
  host (the general affine path is compiled otherwise).  Round 3 folds
  momentum and (h-x)*scale into the output.

Inter-core exchange (the critical path): rows-only fp32 payload, per
boundary phases {0},{1},{2,3} then {0,1},{2},{3}, each AllGather fired
the moment its slots' updates are staged.  The phase schedule keeps the
collective pipeline 100%% busy from ~38us to ~547us (zero gaps), with
each round's compute overlapped under it:
  * score strips over already-arrived blocks are emitted before rebuilds
    of still-in-flight blocks (slot 3's early strips run during the
    phase waits);
  * back-DMAs (collective -> hrows) are deferred into the NEXT round's
    schedule at exactly the hT-build that consumes them: emitted any
    earlier they wait on their collective while blocking later traffic
    (mask loads, staging) on the same DMA queue;
  * scheduler fences (tc.no_sync_barrier) at round boundaries and before
    phase-gated rebuilds stop the static scheduler from hoisting
    collective-gated work ahead of ready work in the per-engine queues
    (priority inversions measured 30-90us each);
  * NOTHING computes on the Pool queue: collectives live there, and any
    instruction queued behind one waits out its full modeled duration.

All scalar parameters (sigmoid/softplus of the inputs) are applied on
the host into small input tensors, so the device program depends only
on shapes (and the gain/bias triviality flag).
"""

import math
import numpy as np

import concourse.bacc as bacc
import concourse.bass as bass
import concourse.mybir as mybir
import concourse.tile as tile
from concourse import bass_utils
from concourse.alu_op_type import AluOpType

F32 = mybir.dt.float32
F32R = mybir.dt.float32r
BF16 = mybir.dt.bfloat16
AF = mybir.ActivationFunctionType
BF16_NP = mybir.dt.np(BF16)

NEG_MASK = -3.0e38  # additive causal mask value (bf16-representable)
NEG_CLAMP = -1.0e29  # threshold clamp: above mask, below any real score

K_SCHEDULE = (4, 8, 16)


class Cfg:
    def __init__(self, B=2, T=2048, D=1024, G=4, S=4, bf16=False, affine=False):
        self.B, self.T, self.D, self.G, self.S = B, T, D, G, S
        self.bf16 = bf16
        self.affine = affine  # general gain/bias path
        self.P = 128
        self.DC = D // 128          # d-chunks
        self.NBLK = G * S           # row blocks per batch
        assert self.NBLK * 128 == T
        self.n_cores = B * G
        self.R = len(K_SCHEDULE)
        # slot k covers j-chunks [0, g(k)); block of core cc in slot k is cc + G*k
        self.g = [G * (k + 1) for k in range(S)]
        self.OFF = [128 * sum(self.g[:k]) for k in range(S)]  # mask free-dim offsets
        self.MTOT = 128 * sum(self.g)
        self.groups = [list(range(b * G, (b + 1) * G)) for b in range(B)]


def build_program(cfg: Cfg):
    """Build the single SPMD Bass/Tile program (identical on all cores)."""
    nc = bacc.Bacc(
        "TRN2", target_bir_lowering=False, debug=False,
        num_devices=cfg.n_cores,
    )
    P, D, T, DC, S, G, R = cfg.P, cfg.D, cfg.T, cfg.DC, cfg.S, cfg.G, cfg.R
    # CD: dtype of the shared h state and all matmul operands.  Plain fp32
    # (4 PE cycles/row) is required for exactness: f32r (1 cycle/row) rounds
    # operands to ~10-bit mantissa on real HW, which flips top-k selections
    # vs the fp32 reference (measured 3.6e-2 rel, over the 2e-2 gate); bf16
    # is worse still (6e-2).  The score/agg matmuls are hidden under the
    # AllGather chain, so the 4x PE cost is mostly free.
    CD = BF16 if cfg.bf16 else F32
    TD = BF16 if cfg.bf16 else F32

    def mmcast(ap):
        return ap

    # ---- I/O ----
    i_hT = nc.dram_tensor("i_hT", [P, DC * T], CD, kind="ExternalInput")
    i_hr = nc.dram_tensor("i_hr", [P, cfg.NBLK * D], CD, kind="ExternalInput")
    i_myh = nc.dram_tensor("i_myh", [P, S * D], F32, kind="ExternalInput")
    i_hTm = nc.dram_tensor("i_hTm", [P, S * D], CD, kind="ExternalInput")
    i_msk = nc.dram_tensor("i_msk", [P, cfg.MTOT], BF16, kind="ExternalInput")
    i_xs = nc.dram_tensor("i_xs", [P, S * D], F32, kind="ExternalInput")
    i_w2 = nc.dram_tensor("i_w2", [P, R * S], F32, kind="ExternalInput")
    # per-partition scalar params: col 0 = mom, 1 = s*(1-mom), 2 = s*mom,
    # 3 = 1-mom, 4+r = mix_r (gelu input scale on the trivial-affine path)
    i_sc = nc.dram_tensor("i_sc", [P, 8], F32, kind="ExternalInput")
    i_idc = nc.dram_tensor("i_idc", [P, 128], CD, kind="ExternalInput")
    i_idf = nc.dram_tensor("i_idf", [P, 128], F32, kind="ExternalInput")
    i_idb = nc.dram_tensor("i_idb", [P, 128], BF16, kind="ExternalInput")
    if cfg.affine:
        i_gm = nc.dram_tensor("i_gm", [R, P, D], F32, kind="ExternalInput")
        i_bb = nc.dram_tensor("i_bb", [R, P, D], F32, kind="ExternalInput")
    o_out = nc.dram_tensor("o_out", [S, P, D], F32, kind="ExternalOutput")

    NH = D // 512  # 512-wide halves of D
    with tile.TileContext(nc) as tc:
        with (
            tc.tile_pool(name="const", bufs=1) as const,
            tc.tile_pool(name="work", bufs=2) as work,
            tc.tile_pool(name="psum", bufs=2, space="PSUM") as psum,
            tc.tile_pool(name="dram", bufs=1, space="DRAM") as dram,
        ):
            # ---- persistent state ----
            hT = const.tile([P, DC * T], CD, name="hT")
            hrows = const.tile([P, cfg.NBLK * D], CD, name="hrows")
            myh = const.tile([P, S * D], F32, name="myh")
            hTm = const.tile([P, S * D], CD, name="hTm")
            w2t = const.tile([P, R * S], F32, name="w2t")
            sct = const.tile([P, 8], F32, name="sct")
            idc = const.tile([P, 128], CD, name="idc")
            idf = const.tile([P, 128], F32, name="idf")
            idb = const.tile([P, 128], BF16, name="idb")

            hTv = hT.rearrange("p (c j) -> p c j", c=DC)
            iTv = i_hT[:].rearrange("p (c j) -> p c j", c=DC)

            # Initial loads in strict first-use order, ONLY what slot-0's
            # chain needs: its staging DMA gates the first phase collective
            # and every later phase chains off it (the collective pipeline
            # is gap-free), so every early microsecond here moves the whole
            # timeline.  All DMA transfers serialize on the one modeled
            # DMA-engines device, so non-critical bytes (rest of hTm/myh,
            # later stripes) are deferred below into the round-0 schedule.
            j1_0 = cfg.g[0]
            nc.sync.dma_start(hTm[:, 0:D // 2], i_hTm[:, 0:D // 2])
            nc.sync.dma_start(hTm[:, D // 2:D], i_hTm[:, D // 2:D])
            # slot-0's first score matmul needs only chunk dc=0 of the hT
            # window; per-dc loads let the PE start ~7us earlier.
            for dc in range(DC):
                nc.sync.dma_start(hTv[:, dc:dc + 1, 0:j1_0 * 128],
                                  iTv[:, dc:dc + 1, 0:j1_0 * 128])
            nc.sync.dma_start(hrows[:, 0:j1_0 * D], i_hr[:, 0:j1_0 * D])
            nc.sync.dma_start(idb[:], i_idb[:])  # gates slot-0's m01 transpose
            nc.sync.dma_start(myh[:, 0:D], i_myh[:, 0:D])
            nc.sync.dma_start(sct[:], i_sc[:])
            w2dma = nc.sync.dma_start(w2t[:], i_w2[:])
            nc.sync.dma_start(idc[:], i_idc[:])
            nc.sync.dma_start(idf[:], i_idf[:])
            # Warm the ACT Gelu table at t~0: the 1.28us table load otherwise
            # lands on slot-0's critical path, which gates the first phase
            # collective and with it the whole gap-free collective chain.
            warm = const.tile([P, 1], F32, name="warm")
            nc.vector.memset(warm[:], 0.0)
            nc.scalar.activation(warm[:], warm[:], AF.Gelu)

            def rest_loads():
                nc.sync.dma_start(hTm[:, D:S * D], i_hTm[:, D:S * D])
                nc.sync.dma_start(myh[:, D:S * D], i_myh[:, D:S * D])

            # Bulk hT/hrows loads are NOT on the Pool queue: Pool DMAs hold
            # the Pool engine for the whole transfer in the cost model, and
            # the first phase collective (also on Pool) would queue behind
            # all 38us of them, delaying the whole (chain-bound) collective
            # pipeline.  They ride the scalar HWDGE queue instead.  The
            # first one is gated on the last small critical load: all DMA
            # transfers serialize on the one modeled DMA-engines device and
            # Tile otherwise hoists these dep-free issues to t~0, starving
            # slot-0's staging (which gates the whole collective chain).
            from concourse.tile import add_dep_helper as _add_dep

            def bulk_load(k, gate=None):
                j0, j1 = cfg.g[k - 1], cfg.g[k]
                d1 = nc.scalar.dma_start(hTv[:, :, j0 * 128:j1 * 128],
                                         iTv[:, :, j0 * 128:j1 * 128])
                d2 = nc.scalar.dma_start(hrows[:, j0 * D:j1 * D],
                                         i_hr[:, j0 * D:j1 * D])
                if gate is not None:
                    for d_ in (d1, d2):
                        _add_dep(d_.ins, gate.ins, sync=True,
                                 reason="bulk loads start after slot-0 critical loads")

            ap_mom = sct[:, 0:1]
            ap_s1m = sct[:, 1:2]
            ap_sm = sct[:, 2:3]
            ap_1m = sct[:, 3:4]

            # per-round, per-phase AllGather buffers (DRAM), rows-only payload.
            # Boundary 0 ships slots {0,1} then {2,3}; boundary 1 (feeding the
            # final round) splits the tail phases so the last round's slot-2/3
            # chains start as soon as their own blocks arrive.
            PHS = [[[0], [1], [2, 3]], [[0, 1], [2], [3]]][:R - 1]
            ag_in = [[dram.tile([len(ph), P, D], CD, name=f"ag_in{r}_{p}", tag=f"agi{r}_{p}")
                      for p, ph in enumerate(PHS[r])] for r in range(R - 1)]
            ag_out = [[dram.tile([len(ph) * G, P, D], CD, name=f"ag_out{r}_{p}", tag=f"ago{r}_{p}")
                       for p, ph in enumerate(PHS[r])] for r in range(R - 1)]
            # slot -> (phase, q) per boundary
            PQ = [{k: (p, q) for p, ph in enumerate(phs) for q, k in enumerate(ph)}
                  for phs in PHS]

            # alternate psum->sbuf copy engines to balance DVE/ACT load
            _alt = [0]
            pending_backs = [{}]  # slot -> deferred back-DMA emitter

            def copy_out(dst, src):
                _alt[0] ^= 1
                if _alt[0]:
                    nc.vector.tensor_copy(dst, src)
                else:
                    nc.scalar.activation(dst, src, AF.Copy)

            for r in range(R):
                K = K_SCHEDULE[r]

                # propagation of slot k-1 is deferred until slot k's scores are
                # queued (the h'-transposes wait on the DVE/ACT elementwise
                # chain and would otherwise stall the next slot's matmuls on
                # the strict PE FIFO).
                pending_prop = None

                def build_blocks(b0, b1, rr=r):
                    # PE-transpose hrows blocks [b0,b1) into hT columns
                    for blk in range(b0, b1):
                        for half in range(2):
                            ptr = psum.tile([P, 512], CD, tag="pt", bufs=2,
                                            name=f"ptr_{rr}_{blk}_{half}")
                            for i4 in range(4):
                                dc = half * 4 + i4
                                nc.tensor.transpose(
                                    ptr[:, i4 * 128:(i4 + 1) * 128],
                                    hrows[:, blk * D + dc * 128: blk * D + (dc + 1) * 128],
                                    idc[:])
                            dst = hTv[:, half * 4:half * 4 + 4,
                                      blk * 128:(blk + 1) * 128]
                            src = ptr[:].rearrange("p (c j) -> p c j", c=4)
                            copy_out(dst, src)

                scs = {}

                def ensure_sc(kk, rr=r):
                    if kk not in scs:
                        W = cfg.g[kk] * 128
                        scs[kk] = work.tile([P, W], F32, tag="sc", bufs=2,
                                            name=f"sc_{rr}_{kk}")

                def score_strips(kk, w0_list, rr=r):
                    ensure_sc(kk)
                    for w0 in w0_list:
                        # per-strip causal mask chunk (small rotating bufs)
                        mskt = work.tile([P, 512], BF16, tag="msk",
                                         bufs=3 if not cfg.affine else 2,
                                         name=f"msk_{rr}_{kk}_{w0}")
                        nc.scalar.dma_start(
                            mskt[:], i_msk[:, cfg.OFF[kk] + w0:cfg.OFF[kk] + w0 + 512])
                        ps = psum.tile([P, 512], F32, tag="ps_sc", bufs=2,
                                       name=f"ps_{rr}_{kk}_{w0}")
                        for dc in range(DC):
                            nc.tensor.matmul(
                                ps[:],
                                mmcast(hTm[:, (kk * DC + dc) * 128:(kk * DC + dc + 1) * 128]),
                                mmcast(hT[:, dc * T + w0: dc * T + w0 + 512]),
                                start=(dc == 0), stop=(dc == DC - 1),
                            )
                        # masked copy PSUM -> SBUF: sc = S + mask (0 / -3e38)
                        nc.vector.scalar_tensor_tensor(
                            scs[kk][:, w0:w0 + 512], ps[:], 1.0, mskt[:],
                            AluOpType.mult, AluOpType.add,
                        )

                def seg16_into(dst16, ap, W, tagsfx, rr=r):
                    """Top-16 candidates of `ap` ([P, W<=512]) into dst16
                    ([P, 16]): top-8, then ranks 9-16 via match_replace.  The
                    scratch rides the shared bufs=1 [P, 512] "scr" tag
                    (sequential reuses serialize via WAR)."""
                    nc.vector.max(out=dst16[:, 0:8], in_=ap)
                    scr = work.tile([P, W], F32, tag="scr",
                                    bufs=1, name=f"scr_{rr}_{tagsfx}")
                    nc.vector.match_replace(out=scr[:], in_to_replace=dst16[:, 0:8],
                                            in_values=ap, imm_value=NEG_MASK)
                    nc.vector.max(out=dst16[:, 8:16], in_=scr[:])

                def top16_th(sc_ap, W, tagsfx, rr=r):
                    """16th-largest source AP over a [P, W] score range,
                    per-512-segment candidates then a merge pass — a flat
                    match_replace over W=2048 would need an 8KB/partition
                    scratch; segments cap it at 2KB."""
                    nseg = (W + 511) // 512
                    candc = work.tile([P, nseg * 16], F32, tag="candc",
                                      bufs=1, name=f"candc_{rr}_{tagsfx}")
                    for s_ in range(nseg):
                        lo = s_ * 512
                        seg16_into(candc[:, s_ * 16:(s_ + 1) * 16],
                                   sc_ap[:, lo:min(lo + 512, W)],
                                   min(512, W - lo), f"{tagsfx}{s_}")
                    if nseg == 1:
                        return candc[:, 8:16]
                    mg = work.tile([P, 16], F32, tag="mg", bufs=1,
                                   name=f"mg_{rr}_{tagsfx}")
                    seg16_into(mg[:], candc[:], nseg * 16, f"{tagsfx}m")
                    return mg[:, 8:16]

                hb_pre = {}
                _hbalt = [0]

                def hb_copy(jc, h_, rr=r):
                    """fp32 hrows chunk -> rotating bf16 tile (the rounding
                    producer the BIR verifier wants for bf16 matmuls).
                    DVE-heavy 2:1 engine split: the modeled DVE copy is
                    0.39us vs ACT's 0.61us for [128, 512]."""
                    hb = work.tile([P, 512], BF16, tag="hb", bufs=6,
                                   name=f"hb_{rr}_{jc}_{h_}")
                    hop = hrows[:, jc * D + h_ * 512: jc * D + h_ * 512 + 512]
                    _hbalt[0] = (_hbalt[0] + 1) % 3
                    if _hbalt[0] == 0:
                        nc.scalar.activation(hb[:], hop, AF.Copy)
                    else:
                        nc.vector.tensor_copy(hb[:], hop)
                    return hb

                def slot_rest(k, rr=r, th_src_in=None, fast_agg=False):
                    """topk -> mask -> aggregate -> update -> defer prop."""
                    nonlocal pending_prop
                    g = cfg.g[k]
                    W = g * 128
                    sc = scs[k]
                    # ---- top-K threshold ----
                    if th_src_in is not None:
                        th_src = th_src_in
                    elif K <= 8:
                        mx = work.tile([P, 8], F32, tag="mx", name=f"mx_{rr}_{k}")
                        nc.vector.max(out=mx[:], in_=sc[:])
                        th_src = mx[:, K - 1:K]
                    else:
                        t16 = top16_th(sc, W, f"f{k}")
                        th_src = t16[:, K - 9:K - 8]
                    th = work.tile([P, 1], F32, tag="th", name=f"th_{rr}_{k}")
                    nc.vector.tensor_scalar_max(th[:], th_src, NEG_CLAMP)
                    # ---- 0/1 neighbor mask.  MUST NOT ride the Pool queue:
                    # collectives live there, and any instruction queued
                    # behind one waits for its (120us) completion. ----
                    m01 = work.tile([P, W], BF16, tag="m01", bufs=1, name=f"m01_{rr}_{k}")
                    nc.vector.tensor_scalar(m01[:], sc[:], th[:], None, AluOpType.is_ge)
                    # ---- transpose mask chunks + aggregate (interleaved):
                    # msg_raw = M01 @ hrows, accumulated 4 j-chunks at a time
                    # through a small rotating transposed-mask tile ----
                    pss = [psum.tile([P, 512], F32, tag="ps_ag", bufs=2,
                                     name=f"pa_{rr}_{k}_{h_}") for h_ in range(NH)]
                    for jq in range(0, g, 4):
                        ptm = psum.tile([P, 512], BF16, tag="ptb", bufs=2,
                                        name=f"ptm_{rr}_{k}_{jq}")
                        for i4 in range(4):
                            nc.tensor.transpose(
                                ptm[:, i4 * 128:(i4 + 1) * 128],
                                m01[:, (jq + i4) * 128:(jq + i4 + 1) * 128], idb[:])
                        # fast_agg (last round, tail slot only): bf16
                        # aggregation at 1 PE cycle/row instead of fp32's 4.
                        # The mask operand is 0/1 (exact); the h operand is
                        # rounded to bf16 by ACT/DVE chunk copies (the BIR
                        # verifier requires low-precision matmul operands to
                        # come from rounding producers, so no bitcast).
                        # Aggregation error here reaches only the OUTPUT
                        # (~1e-4 rel, no further top-k rounds to flip);
                        # earlier rounds stay fp32 — their msg error would
                        # perturb the next round's scores ~45x and flip
                        # selections.
                        mdt = BF16 if fast_agg else CD
                        mts = work.tile([P, 512], mdt, tag="mt",
                                        bufs=2 if not cfg.affine else 1,
                                        name=f"mt_{rr}_{k}_{jq}")
                        copy_out(mts[:], ptm[:])
                        for h_ in range(NH):
                            for i4 in range(4):
                                jc = jq + i4
                                hop = hrows[:, jc * D + h_ * 512: jc * D + h_ * 512 + 512]
                                if fast_agg:
                                    hb = hb_pre.pop((jc, h_), None)
                                    if hb is None:
                                        hb = hb_copy(jc, h_)
                                    hop = hb[:]
                                nc.tensor.matmul(
                                    pss[h_][:], mmcast(mts[:, i4 * 128:(i4 + 1) * 128]),
                                    mmcast(hop),
                                    start=(jc == 0), stop=(jc == g - 1))
                    # ---- elementwise update ----
                    w2ap = w2t[:, rr * S + k: rr * S + k + 1]
                    for h_ in range(NH):
                        sl = slice(k * D + h_ * 512, k * D + (h_ + 1) * 512)
                        hsl = slice(h_ * 512, (h_ + 1) * 512)
                        t1 = work.tile([P, 512], F32, tag="t1",
                                       bufs=2 if not cfg.affine else 1,
                                       name=f"t1_{rr}_{k}_{h_}")
                        nc.vector.scalar_tensor_tensor(
                            t1[:], pss[h_][:], w2ap, myh[:, sl],
                            AluOpType.mult, AluOpType.add)
                        if cfg.affine:
                            gmt = work.tile([P, 512], F32, tag="gmt", bufs=2,
                                            name=f"gmt_{rr}_{k}_{h_}")
                            nc.scalar.dma_start(gmt[:], i_gm[rr][:, hsl])
                            bbt = work.tile([P, 512], F32, tag="bbt", bufs=2,
                                            name=f"bbt_{rr}_{k}_{h_}")
                            nc.scalar.dma_start(bbt[:], i_bb[rr][:, hsl])
                            nc.vector.tensor_mul(t1[:], t1[:], gmt[:])
                            nc.vector.tensor_add(t1[:], t1[:], bbt[:])
                        gl = work.tile([P, 512], F32, tag="gl",
                                       bufs=2 if not cfg.affine else 1,
                                       name=f"gl_{rr}_{k}_{h_}")
                        if cfg.affine:
                            nc.scalar.activation(gl[:], t1[:], AF.Gelu)
                        else:
                            # u = (psum*w2 + h) * mix  (w2 carries the /mix)
                            nc.scalar.activation(gl[:], t1[:], AF.Gelu,
                                                 scale=sct[:, 4 + rr:5 + rr])
                        if rr < R - 1:
                            nc.vector.tensor_scalar_mul(gl[:], gl[:], ap_1m)
                            # in-place: myh = mom*myh + (1-mom)*gelu
                            nc.vector.scalar_tensor_tensor(
                                myh[:, sl], myh[:, sl], ap_mom, gl[:],
                                AluOpType.mult, AluOpType.add)
                        else:
                            xst = work.tile([P, 512], F32, tag="xst", bufs=1,
                                            name=f"xst_{rr}_{k}_{h_}")
                            nc.sync.dma_start(xst[:], i_xs[:, sl])
                            # gl <- s*(1-mom)*gelu - s*x
                            nc.vector.scalar_tensor_tensor(
                                gl[:], gl[:], ap_s1m, xst[:],
                                AluOpType.mult, AluOpType.subtract)
                            # t1 <- s*mom*h + gl
                            nc.vector.scalar_tensor_tensor(
                                t1[:], myh[:, sl], ap_sm, gl[:],
                                AluOpType.mult, AluOpType.add)
                            nc.sync.dma_start(o_out[k, :, hsl], t1[:])
                    # ---- propagate h' (transposed into hTm + staged rows) ----
                    if rr < R - 1:
                        def _prop(kk=k):
                            if cfg.bf16:
                                hi_t = work.tile([P, D], CD, tag="hi_t", bufs=1,
                                                 name=f"hi_{rr}_{kk}")
                                nc.scalar.activation(
                                    hi_t[:], myh[:, kk * D:(kk + 1) * D], AF.Copy)
                                src = hi_t[:]
                            else:
                                src = myh[:, kk * D:(kk + 1) * D]
                            for half in range(2):
                                pto = psum.tile([P, 512], TD, tag="pt", bufs=2,
                                                name=f"pto_{rr}_{kk}_{half}")
                                for i4 in range(4):
                                    dc = half * 4 + i4
                                    nc.tensor.transpose(
                                        pto[:, i4 * 128:(i4 + 1) * 128],
                                        src[:, dc * 128:(dc + 1) * 128],
                                        idc[:] if cfg.bf16 else idf[:])
                                copy_out(hTm[:, (kk * DC + half * 4) * 128:
                                              (kk * DC + half * 4 + 4) * 128], pto[:])
                            p_, q_ = PQ[rr][kk]
                            nc.sync.dma_start(ag_in[rr][p_][q_], src)
                            # fire the phase collective as soon as all its
                            # slots are staged; it only touches DRAM, so it
                            # overlaps the remaining slots' compute.
                            if q_ == len(PHS[rr][p_]) - 1 and p_ < len(PHS[rr]) - 1:
                                nc.gpsimd.collective_compute(
                                    "AllGather", AluOpType.bypass,
                                    replica_groups=cfg.groups,
                                    ins=[ag_in[rr][p_].opt()],
                                    outs=[ag_out[rr][p_].opt()])
                        pending_prop = _prop

                def fire_prop():
                    nonlocal pending_prop
                    if pending_prop is not None:
                        with tc.high_priority():
                            pending_prop()
                        pending_prop = None

                # ---- the round's emission schedule.  The PE FIFO is strict,
                # so emission order = execution order: score strips over
                # already-built hT blocks are emitted before rebuilds whose
                # back-DMA may still be waiting on a phase collective, and
                # slot 3's early strips run during the phase-1 wait.
                if r == 0:
                    for k in range(S):
                        score_strips(k, range(0, cfg.g[k] * 128, 512))
                        fire_prop()
                        slot_rest(k)
                        if k == 0:
                            rest_loads()
                        if k < S - 1:
                            # hT/hrows for the NEXT slot
                            bulk_load(k + 1, gate=w2dma)
                else:
                    # scheduler fence at the round boundary: without it the
                    # static scheduler interleaves this round's DVE/ACT ops
                    # before the previous round's tail updates in the per-
                    # engine queues, stalling the staging of the boundary
                    # collectives behind collective-gated work.
                    tc.no_sync_barrier()

                    def fire_backs(at_slot):
                        em = pending_backs[0].pop(at_slot, None)
                        if em is not None:
                            em()

                    fire_backs(0)
                    build_blocks(0, G)
                    score_strips(0, [0])
                    slot_rest(0)
                    score_strips(1, [0])
                    fire_backs(1)
                    build_blocks(G, 2 * G)
                    score_strips(1, [512])
                    fire_prop()  # prop(0): stages the next boundary's phase 0
                    slot_rest(1)
                    score_strips(2, [0, 512])
                    fire_prop()  # prop(1) EARLY: its staging must not queue
                    #              behind the phase-dependent strips below
                    score_strips(3, [0, 512])
                    tc.no_sync_barrier()  # keep the phase-gated rebuild below
                    #                       from being scheduled before the
                    #                       ready work above
                    fire_backs(2)
                    build_blocks(2 * G, 3 * G)
                    score_strips(2, [1024])
                    score_strips(3, [1024])
                    slot_rest(2)
                    fire_prop()  # prop(2) early, same reason
                    if r == R - 1:
                        # Prefix top-16 candidates over the already-computed
                        # [0, 1536) scores, emitted AFTER slot-2's DVE chain
                        # so it doesn't push slot-2's (zero-slack) work into
                        # the tail.  Runs hidden under the last collective.
                        candt = work.tile([P, 64], F32, tag="candc", bufs=1,
                                          name=f"candt_{r}_3")
                        for s_ in range(3):
                            seg16_into(candt[:, s_ * 16:(s_ + 1) * 16],
                                       scs[3][:, s_ * 512:(s_ + 1) * 512],
                                       512, f"p3{s_}")
                    tc.no_sync_barrier()
                    fire_backs(3)
                    if r < R - 1:
                        build_blocks(3 * G, 4 * G)
                        score_strips(3, [1536])
                        slot_rest(3)
                    else:
                        # ---- last-round tail: everything after the final
                        # collective is serial program end, so the stripe-3
                        # work is pipelined per BLOCK (each block's hT
                        # rebuild + 128-wide score segment fires as soon as
                        # its own rank's back-DMA lands) and the top-16
                        # threshold merges the hidden prefix candidates with
                        # candidates from the just-arrived 512-wide suffix
                        # instead of re-scanning all 2048 columns. ----
                        ensure_sc(3)
                        w0 = 1536
                        mskt = work.tile([P, 512], BF16, tag="msk", bufs=3,
                                         name=f"msk_{r}_3_tail")
                        nc.scalar.dma_start(
                            mskt[:], i_msk[:, cfg.OFF[3] + w0:cfg.OFF[3] + w0 + 512])
                        ps = psum.tile([P, 512], F32, tag="ps_sc", bufs=2,
                                       name=f"ps_{r}_3_tail")
                        # builds and strip segments interleaved one build
                        # ahead: each build's psum->SBUF copy costs a DVE
                        # round-trip, which the PREVIOUS block's strip
                        # matmuls hide.
                        def strip_seg(q):
                            for dc in range(DC):
                                nc.tensor.matmul(
                                    ps[:, q * 128:(q + 1) * 128],
                                    mmcast(hTm[:, (3 * DC + dc) * 128:(3 * DC + dc + 1) * 128]),
                                    mmcast(hT[:, dc * T + w0 + q * 128:
                                              dc * T + w0 + (q + 1) * 128]),
                                    start=(dc == 0), stop=(dc == DC - 1),
                                )
                        build_blocks(3 * G, 3 * G + 1)
                        build_blocks(3 * G + 1, 3 * G + 2)
                        strip_seg(0)
                        build_blocks(3 * G + 2, 3 * G + 3)
                        strip_seg(1)
                        build_blocks(3 * G + 3, 3 * G + 4)
                        strip_seg(2)
                        strip_seg(3)
                        nc.vector.scalar_tensor_tensor(
                            scs[3][:, w0:w0 + 512], ps[:], 1.0, mskt[:],
                            AluOpType.mult, AluOpType.add,
                        )
                        for (jc_, h2_) in [(0, 0), (1, 0), (2, 0),
                                           (3, 0), (0, 1), (1, 1)]:
                            hb_pre[(jc_, h2_)] = hb_copy(jc_, h2_)
                        seg16_into(candt[:, 48:64], scs[3][:, w0:w0 + 512],
                                   512, "s3")
                        mgt = work.tile([P, 16], F32, tag="mg", bufs=1,
                                        name=f"mgt_{r}_3")
                        seg16_into(mgt[:], candt[:], 64, "m3")
                        slot_rest(3, th_src_in=mgt[:, 15:16], fast_agg=True)

                fire_prop()  # prop(3)

                # ---- round boundary: last phase collective + back-DMAs ----
                # Back-DMAs are emitted after the slot loop so Tile orders
                # them after this round's readers of hrows.  The fence keeps
                # the scheduler from hoisting them ahead of this round's tail
                # ops on the shared DMA queues (priority inversion: a back-DMA
                # WAR-waits on this round's aggregation, which itself needs a
                # psum copy queued behind that same back-DMA).
                if r < R - 1:
                    tc.no_sync_barrier()
                    nph = len(PHS[r])
                    nc.gpsimd.collective_compute(
                        "AllGather", AluOpType.bypass, replica_groups=cfg.groups,
                        ins=[ag_in[r][nph - 1].opt()], outs=[ag_out[r][nph - 1].opt()])

                    def _backs(p_, eng, rr=r):
                        # one DMA per (phase, q, rank): finer pieces let the
                        # tail's per-block hT rebuilds start as soon as THEIR
                        # rank's rows land instead of waiting out one big
                        # strided transfer on the serial DMA device.
                        nq = len(PHS[rr][p_])
                        srcv = ag_out[rr][p_][:].rearrange(
                            "(rnk q) p d -> q rnk p d", q=nq)
                        for q in range(nq):
                            base = PHS[rr][p_][q] * G
                            for rnk in range(G):
                                for hf in range(2):
                                    eng.dma_start(
                                        hrows[:, (base + rnk) * D + hf * (D // 2):
                                               (base + rnk) * D + (hf + 1) * (D // 2)],
                                        srcv[q, rnk, :, hf * (D // 2):(hf + 1) * (D // 2)])

                    # ALL back-DMAs are deferred into the next round's
                    # schedule (sync queue), each emitted just before the
                    # hT rebuild that consumes it: emitted any earlier they
                    # sit on a DMA queue WAITING on their phase collective,
                    # blocking later traffic on that queue (mask loads,
                    # staging) that the next round needs much sooner.
                    pending_backs[0] = {
                        PHS[r][p_][0]: (lambda rr=r, pp=p_: _backs(pp, nc.sync, rr))
                        for p_ in range(nph)}

    nc.compile()
    return nc


# ------------------------------------------------------------------
# Host side
# ------------------------------------------------------------------

def _sigmoid(v):
    return 1.0 / (1.0 + math.exp(-float(v)))


def prep_inputs(cfg: Cfg, x, gain, bias, log_mix, log_momentum, log_scale):
    """Build the per-core input maps (numpy)."""
    P, D, T, DC, S, G, R = cfg.P, cfg.D, cfg.T, cfg.DC, cfg.S, cfg.G, cfg.R
    cd_np = BF16_NP if cfg.bf16 else np.float32
    x = np.asarray(x, np.float32)
    gain = np.asarray(gain, np.float32)
    bias = np.asarray(bias, np.float32)
    mix = np.array([_sigmoid(v) for v in np.asarray(log_mix, np.float32)], np.float64)
    mom = _sigmoid(log_momentum)
    s = math.log1p(math.exp(float(log_scale))) + 0.01

    scl = np.zeros((P, 8), np.float32)
    scl[:, 0] = mom
    scl[:, 1] = s * (1.0 - mom)
    scl[:, 2] = s * mom
    scl[:, 3] = 1.0 - mom
    scl[:, 4:4 + R] = mix.astype(np.float32)[None, :]
    idc = np.eye(128, dtype=cd_np)

    common = {"i_sc": scl, "i_idc": idc, "i_idb": np.eye(128, dtype=BF16_NP),
              "i_idf": np.eye(128, dtype=np.float32)}
    if cfg.affine:
        common["i_gm"] = np.ascontiguousarray(np.broadcast_to(
            (gain * mix[:, None].astype(np.float32)).astype(np.float32)[:, None, :],
            (R, P, D)))
        common["i_bb"] = np.ascontiguousarray(np.broadcast_to(bias[:, None, :], (R, P, D)))

    in_maps = []
    for c in range(cfg.n_cores):
        b, cc = divmod(c, G)
        blocks = [cc + G * k for k in range(S)]
        h0 = x[b]  # [T, D]
        hT0 = np.ascontiguousarray(
            h0.T.reshape(DC, 128, T).transpose(1, 0, 2)).reshape(128, DC * T).astype(cd_np)
        hr0 = np.ascontiguousarray(
            h0.astype(cd_np).reshape(cfg.NBLK, 128, D).transpose(1, 0, 2)).reshape(128, cfg.NBLK * D)
        hb = h0.reshape(cfg.NBLK, 128, D)[blocks]  # [S,128,D]
        myh0 = np.ascontiguousarray(hb.transpose(1, 0, 2)).reshape(128, S * D)
        hTm0 = np.ascontiguousarray(
            hb.reshape(S, 128, DC, 128).transpose(3, 0, 2, 1)).reshape(128, S * D).astype(cd_np)
        msk = np.zeros((P, cfg.MTOT), BF16_NP)
        for k in range(S):
            W = cfg.g[k] * 128
            rowid = blocks[k] * 128 + np.arange(128)  # [128]
            j = np.arange(W)
            m = np.where(j[None, :] <= rowid[:, None], 0.0, NEG_MASK).astype(BF16_NP)
            msk[:, cfg.OFF[k]:cfg.OFF[k] + W] = m
        w2 = np.zeros((P, R * S), np.float32)
        for r in range(R):
            for k in range(S):
                cnt = np.minimum(blocks[k] * 128 + np.arange(128) + 1, K_SCHEDULE[r])
                w2[:, r * S + k] = ((1.0 - mix[r]) / (mix[r] * cnt)).astype(np.float32)
        in_maps.append(dict(common,
                            i_hT=hT0, i_hr=hr0, i_myh=myh0, i_hTm=hTm0,
                            i_msk=msk, i_xs=(s * myh0.astype(np.float64)).astype(np.float32),
                            i_w2=w2))
    return in_maps


def assemble_output(cfg: Cfg, results, dtype=np.float32):
    """results: list (per core) of {'o_out': [S,128,D]} -> full [B,T,D]."""
    out = np.zeros((cfg.B, cfg.T, cfg.D), dtype)
    for c in range(cfg.n_cores):
        b, cc = divmod(c, cfg.G)
        o = results[c]["o_out"]
        for k in range(cfg.S):
            blk = cc + cfg.G * k
            out[b, blk * 128:(blk + 1) * 128] = o[k]
    return out


_PROGRAM_CACHE = {}


def _get_program(cfg: Cfg):
    key = (cfg.B, cfg.T, cfg.D, cfg.G, cfg.S, cfg.bf16, cfg.affine)
    if key not in _PROGRAM_CACHE:
        _PROGRAM_CACHE[key] = build_program(cfg)
    return _PROGRAM_CACHE[key]


def run(cfg: Cfg, inputs: dict, trace: bool = False):
    nc = _get_program(cfg)
    in_maps = prep_inputs(cfg, **inputs)
    res = bass_utils.run_bass_kernel_spmd(
        nc, in_maps, list(range(cfg.n_cores)), trace=trace)
    out = assemble_output(cfg, res.results)
    return out, res


def kernel(**inputs) -> np.ndarray:
    trivial = (np.all(np.asarray(inputs["gain"]) == 1.0)
               and np.all(np.asarray(inputs["bias"]) == 0.0))
    cfg = Cfg(affine=not trivial)  # B=2, T=2048, D=1024, 8 cores
    out, _ = run(cfg, inputs)
    return out.astype(np.float32)



# revision 42
# speedup vs baseline: 1.0155x; 1.0013x over previous
"""Trainium2 Bass kernel for nn_DGN6 (gnn_message_passing).

Reference computation (per batch element, 3 rounds with K = 4, 8, 16):
    S = h @ h.T; causal top-K neighbors per row; msg = masked mean of
    neighbor rows; h = mom*h + (1-mom)*gelu((mix*h + (1-mix)*msg)*gain + bias)
Output: (h - x) * scale.

Distribution: data-parallel over B (2 batches), each batch's rows split
over 4 cores (8 cores total).  Core c handles batch c//4 and, within it,
4 row-blocks of 128 rows: blocks {cc + 4k, k=0..3} where cc = c%4 ("slot"
k holds block cc + 4k).  Every core runs an IDENTICAL instruction stream
(one SPMD program); per-core differences live entirely in input DATA
(causal masks, row data, per-row weights).

Numerics (all learned the hard way, measured on HW):
  * Scores and all exchanged h state stay fp32.  The reference's top-K
    selection sits on a cliff: quantizing h to bf16 (6e-2), f32r/tf32
    (3.6e-2), or even bf16+fp8-residual (5e-2) flips near-tied
    selections and single flips cost ~0.05 absolute error -- over the
    2e-2 gate.  fp32 measures 3e-7.
  * ONE exception: the LAST round's tail-slot aggregation runs in bf16
    (1 PE cycle/row vs 4).  Its error reaches only the output (no more
    top-k rounds downstream): measured 7.6e-5 rel, 260x under the gate.
    The bf16 operands come from ACT/DVE rounding copies because the BIR
    verifier rejects low-precision matmuls fed by non-rounding
    producers (DMAs/bitcasts).
  * fp32 matmuls cost 4 PE cycles/row, but the timeline is bound by the
    AllGather pipeline (cost model: 15us + out_bytes/40GBps, serialized
    per core), so most of the PE cost hides under it.

Timeline (cost model): first phase collective fires at ~31.5us (gated
by slot-0's round-0 staging: every early microsecond here shifts the
whole gap-free chain), chain busy 31.5..541 with zero gaps, then a
~37us tail = per-rank back-DMA pieces -> pipelined per-block hT build +
128-wide score segments -> prefix/suffix-merged top-16 (prefix
candidates precomputed under the last collective) -> bf16 aggregation
with prefetched rounding copies -> quarter-split elementwise + output.
remote_dma / remote_dma_broadcast (SBUF-to-SBUF, ~180GB/s modeled vs
the collective's 40GB/s) would cut the chain 5-10x but faults the axon
sandbox's exec unit (probed: desc-gen + trigger of even a sem-only
remote update dies; plain ucode ext-ISA works), so collectives it is.

Layout/dataflow per round, per slot:
  hT-build (rounds>0) PE-transposes hrows blocks into hT right before
  the score strips that need them; scores accumulate 512-wide strips
  (8 d-chunk matmuls each) into PSUM, masked-copied to SBUF with an
  additive causal mask (0/-3e38, per-strip streamed tiles); top-K via
  vector.max (+match_replace+max for K=16) -> threshold; M01 = (S>=th)
  bf16; M01 chunks PE-transposed 4-at-a-time through a rotating tile and
  immediately consumed by the aggregation matmuls (fp32, PSUM-accumulated
  over causal j-chunks); update u = (msg*w2 + h), h' = mom*h +
  (1-mom)*gelu(u*mix) with mix applied via the ACT engine's input scale
  (w2 = (1-mix)/(mix*cnt) per row); gain==1/bias==0 is detected on the
  host (the general affine path is compiled otherwise).  Round 3 folds
  momentum and (h-x)*scale into the output.

Inter-core exchange (the critical path): rows-only fp32 payload, per
boundary phases {0},{1},{2,3} then {0,1},{2},{3}, each AllGather fired
the moment its slots' updates are staged.  The phase schedule keeps the
collective pipeline 100%% busy from ~31.5us to ~541us (zero gaps), with
each round's compute overlapped under it (phase-count/shape variants
all measured worse: merging b1 {2},{3} trades 15us of chain for 20us
of serialized tail; any phase containing slot 2 can't fire before
~330us because slot-2-r1 needs b0's last phase plus 38us of work):
  * score strips over already-arrived blocks are emitted before rebuilds
    of still-in-flight blocks (slot 3's early strips run during the
    phase waits);
  * back-DMAs (collective -> hrows) are deferred into the NEXT round's
    schedule at exactly the hT-build that consumes them: emitted any
    earlier they wait on their collective while blocking later traffic
    (mask loads, staging) on the same DMA queue;
  * scheduler fences (tc.no_sync_barrier) at round boundaries and before
    phase-gated rebuilds stop the static scheduler from hoisting
    collective-gated work ahead of ready work in the per-engine queues
    (priority inversions measured 30-90us each);
  * NOTHING computes on the Pool queue: collectives live there, and any
    instruction queued behind one waits out its full modeled duration.

All scalar parameters (sigmoid/softplus of the inputs) are applied on
the host into small input tensors, so the device program depends only
on shapes (and the gain/bias triviality flag).
"""

import math
import numpy as np

import concourse.bacc as bacc
import concourse.bass as bass
import concourse.mybir as mybir
import concourse.tile as tile
from concourse import bass_utils
from concourse.alu_op_type import AluOpType

F32 = mybir.dt.float32
F32R = mybir.dt.float32r
BF16 = mybir.dt.bfloat16
AF = mybir.ActivationFunctionType
BF16_NP = mybir.dt.np(BF16)

NEG_MASK = -3.0e38  # additive causal mask value (bf16-representable)
NEG_CLAMP = -1.0e29  # threshold clamp: above mask, below any real score

K_SCHEDULE = (4, 8, 16)


class Cfg:
    def __init__(self, B=2, T=2048, D=1024, G=4, S=4, bf16=False, affine=False):
        self.B, self.T, self.D, self.G, self.S = B, T, D, G, S
        self.bf16 = bf16
        self.affine = affine  # general gain/bias path
        self.P = 128
        self.DC = D // 128          # d-chunks
        self.NBLK = G * S           # row blocks per batch
        assert self.NBLK * 128 == T
        self.n_cores = B * G
        self.R = len(K_SCHEDULE)
        # slot k covers j-chunks [0, g(k)); block of core cc in slot k is cc + G*k
        self.g = [G * (k + 1) for k in range(S)]
        self.OFF = [128 * sum(self.g[:k]) for k in range(S)]  # mask free-dim offsets
        self.MTOT = 128 * sum(self.g)
        self.groups = [list(range(b * G, (b + 1) * G)) for b in range(B)]


def build_program(cfg: Cfg):
    """Build the single SPMD Bass/Tile program (identical on all cores)."""
    nc = bacc.Bacc(
        "TRN2", target_bir_lowering=False, debug=False,
        num_devices=cfg.n_cores,
    )
    P, D, T, DC, S, G, R = cfg.P, cfg.D, cfg.T, cfg.DC, cfg.S, cfg.G, cfg.R
    # CD: dtype of the shared h state and all matmul operands.  Plain fp32
    # (4 PE cycles/row) is required for exactness: f32r (1 cycle/row) rounds
    # operands to ~10-bit mantissa on real HW, which flips top-k selections
    # vs the fp32 reference (measured 3.6e-2 rel, over the 2e-2 gate); bf16
    # is worse still (6e-2).  The score/agg matmuls are hidden under the
    # AllGather chain, so the 4x PE cost is mostly free.
    CD = BF16 if cfg.bf16 else F32
    TD = BF16 if cfg.bf16 else F32

    def mmcast(ap):
        return ap

    # ---- I/O ----
    i_hT = nc.dram_tensor("i_hT", [P, DC * T], CD, kind="ExternalInput")
    i_hr = nc.dram_tensor("i_hr", [P, cfg.NBLK * D], CD, kind="ExternalInput")
    i_myh = nc.dram_tensor("i_myh", [P, S * D], F32, kind="ExternalInput")
    i_hTm = nc.dram_tensor("i_hTm", [P, S * D], CD, kind="ExternalInput")
    i_msk = nc.dram_tensor("i_msk", [P, cfg.MTOT], BF16, kind="ExternalInput")
    i_xs = nc.dram_tensor("i_xs", [P, S * D], F32, kind="ExternalInput")
    i_w2 = nc.dram_tensor("i_w2", [P, R * S], F32, kind="ExternalInput")
    # per-partition scalar params: col 0 = mom, 1 = s*(1-mom), 2 = s*mom,
    # 3 = 1-mom, 4+r = mix_r (gelu input scale on the trivial-affine path)
    i_sc = nc.dram_tensor("i_sc", [P, 8], F32, kind="ExternalInput")
    i_idc = nc.dram_tensor("i_idc", [P, 128], CD, kind="ExternalInput")
    i_idf = nc.dram_tensor("i_idf", [P, 128], F32, kind="ExternalInput")
    i_idb = nc.dram_tensor("i_idb", [P, 128], BF16, kind="ExternalInput")
    if cfg.affine:
        i_gm = nc.dram_tensor("i_gm", [R, P, D], F32, kind="ExternalInput")
        i_bb = nc.dram_tensor("i_bb", [R, P, D], F32, kind="ExternalInput")
    o_out = nc.dram_tensor("o_out", [S, P, D], F32, kind="ExternalOutput")

    NH = D // 512  # 512-wide halves of D
    with tile.TileContext(nc) as tc:
        with (
            tc.tile_pool(name="const", bufs=1) as const,
            tc.tile_pool(name="work", bufs=2) as work,
            tc.tile_pool(name="psum", bufs=2, space="PSUM") as psum,
            tc.tile_pool(name="dram", bufs=1, space="DRAM") as dram,
        ):
            # ---- persistent state ----
            hT = const.tile([P, DC * T], CD, name="hT")
            hrows = const.tile([P, cfg.NBLK * D], CD, name="hrows")
            myh = const.tile([P, S * D], F32, name="myh")
            hTm = const.tile([P, S * D], CD, name="hTm")
            w2t = const.tile([P, R * S], F32, name="w2t")
            sct = const.tile([P, 8], F32, name="sct")
            idc = const.tile([P, 128], CD, name="idc")
            idf = const.tile([P, 128], F32, name="idf")
            idb = const.tile([P, 128], BF16, name="idb")

            hTv = hT.rearrange("p (c j) -> p c j", c=DC)
            iTv = i_hT[:].rearrange("p (c j) -> p c j", c=DC)

            # Initial loads in strict first-use order, ONLY what slot-0's
            # chain needs: its staging DMA gates the first phase collective
            # and every later phase chains off it (the collective pipeline
            # is gap-free), so every early microsecond here moves the whole
            # timeline.  All DMA transfers serialize on the one modeled
            # DMA-engines device, so non-critical bytes (rest of hTm/myh,
            # later stripes) are deferred below into the round-0 schedule.
            j1_0 = cfg.g[0]
            nc.sync.dma_start(hTm[:, 0:D // 2], i_hTm[:, 0:D // 2])
            nc.sync.dma_start(hTm[:, D // 2:D], i_hTm[:, D // 2:D])
            # slot-0's first score matmul needs only chunk dc=0 of the hT
            # window; per-dc loads let the PE start ~7us earlier.
            for dc in range(DC):
                nc.sync.dma_start(hTv[:, dc:dc + 1, 0:j1_0 * 128],
                                  iTv[:, dc:dc + 1, 0:j1_0 * 128])
            nc.sync.dma_start(hrows[:, 0:j1_0 * D], i_hr[:, 0:j1_0 * D])
            nc.sync.dma_start(idb[:], i_idb[:])  # gates slot-0's m01 transpose
            nc.sync.dma_start(myh[:, 0:D], i_myh[:, 0:D])
            nc.sync.dma_start(sct[:], i_sc[:])
            w2dma = nc.sync.dma_start(w2t[:], i_w2[:])
            nc.sync.dma_start(idc[:], i_idc[:])
            nc.sync.dma_start(idf[:], i_idf[:])
            # Warm the ACT Gelu table at t~0: the 1.28us table load otherwise
            # lands on slot-0's critical path, which gates the first phase
            # collective and with it the whole gap-free collective chain.
            warm = const.tile([P, 1], F32, name="warm")
            nc.vector.memset(warm[:], 0.0)
            nc.scalar.activation(warm[:], warm[:], AF.Gelu)

            def rest_loads():
                nc.sync.dma_start(hTm[:, D:S * D], i_hTm[:, D:S * D])
                nc.sync.dma_start(myh[:, D:S * D], i_myh[:, D:S * D])

            # Bulk hT/hrows loads are NOT on the Pool queue: Pool DMAs hold
            # the Pool engine for the whole transfer in the cost model, and
            # the first phase collective (also on Pool) would queue behind
            # all 38us of them, delaying the whole (chain-bound) collective
            # pipeline.  They ride the scalar HWDGE queue instead.  The
            # first one is gated on the last small critical load: all DMA
            # transfers serialize on the one modeled DMA-engines device and
            # Tile otherwise hoists these dep-free issues to t~0, starving
            # slot-0's staging (which gates the whole collective chain).
            from concourse.tile import add_dep_helper as _add_dep

            def bulk_load(k, gate=None):
                j0, j1 = cfg.g[k - 1], cfg.g[k]
                d1 = nc.scalar.dma_start(hTv[:, :, j0 * 128:j1 * 128],
                                         iTv[:, :, j0 * 128:j1 * 128])
                d2 = nc.scalar.dma_start(hrows[:, j0 * D:j1 * D],
                                         i_hr[:, j0 * D:j1 * D])
                if gate is not None:
                    for d_ in (d1, d2):
                        _add_dep(d_.ins, gate.ins, sync=True,
                                 reason="bulk loads start after slot-0 critical loads")

            ap_mom = sct[:, 0:1]
            ap_s1m = sct[:, 1:2]
            ap_sm = sct[:, 2:3]
            ap_1m = sct[:, 3:4]

            # per-round, per-phase AllGather buffers (DRAM), rows-only payload.
            # Boundary 0 ships slots {0,1} then {2,3}; boundary 1 (feeding the
            # final round) splits the tail phases so the last round's slot-2/3
            # chains start as soon as their own blocks arrive.
            PHS = [[[0], [1], [2, 3]], [[0, 1], [2], [3]]][:R - 1]
            ag_in = [[dram.tile([len(ph), P, D], CD, name=f"ag_in{r}_{p}", tag=f"agi{r}_{p}")
                      for p, ph in enumerate(PHS[r])] for r in range(R - 1)]
            ag_out = [[dram.tile([len(ph) * G, P, D], CD, name=f"ag_out{r}_{p}", tag=f"ago{r}_{p}")
                       for p, ph in enumerate(PHS[r])] for r in range(R - 1)]
            # slot -> (phase, q) per boundary
            PQ = [{k: (p, q) for p, ph in enumerate(phs) for q, k in enumerate(ph)}
                  for phs in PHS]

            # alternate psum->sbuf copy engines to balance DVE/ACT load
            _alt = [0]
            pending_backs = [{}]  # slot -> deferred back-DMA emitter

            def copy_out(dst, src):
                _alt[0] ^= 1
                if _alt[0]:
                    nc.vector.tensor_copy(dst, src)
                else:
                    nc.scalar.activation(dst, src, AF.Copy)

            for r in range(R):
                K = K_SCHEDULE[r]

                # propagation of slot k-1 is deferred until slot k's scores are
                # queued (the h'-transposes wait on the DVE/ACT elementwise
                # chain and would otherwise stall the next slot's matmuls on
                # the strict PE FIFO).
                pending_prop = None

                def build_blocks(b0, b1, rr=r):
                    # PE-transpose hrows blocks [b0,b1) into hT columns
                    for blk in range(b0, b1):
                        for half in range(2):
                            ptr = psum.tile([P, 512], CD, tag="pt", bufs=2,
                                            name=f"ptr_{rr}_{blk}_{half}")
                            for i4 in range(4):
                                dc = half * 4 + i4
                                nc.tensor.transpose(
                                    ptr[:, i4 * 128:(i4 + 1) * 128],
                                    hrows[:, blk * D + dc * 128: blk * D + (dc + 1) * 128],
                                    idc[:])
                            dst = hTv[:, half * 4:half * 4 + 4,
                                      blk * 128:(blk + 1) * 128]
                            src = ptr[:].rearrange("p (c j) -> p c j", c=4)
                            copy_out(dst, src)

                scs = {}

                def ensure_sc(kk, rr=r):
                    if kk not in scs:
                        W = cfg.g[kk] * 128
                        scs[kk] = work.tile([P, W], F32, tag="sc", bufs=2,
                                            name=f"sc_{rr}_{kk}")

                def score_strips(kk, w0_list, rr=r):
                    ensure_sc(kk)
                    for w0 in w0_list:
                        # per-strip causal mask chunk (small rotating bufs)
                        mskt = work.tile([P, 512], BF16, tag="msk",
                                         bufs=2,
                                         name=f"msk_{rr}_{kk}_{w0}")
                        nc.scalar.dma_start(
                            mskt[:], i_msk[:, cfg.OFF[kk] + w0:cfg.OFF[kk] + w0 + 512])
                        ps = psum.tile([P, 512], F32, tag="ps_sc", bufs=2,
                                       name=f"ps_{rr}_{kk}_{w0}")
                        for dc in range(DC):
                            nc.tensor.matmul(
                                ps[:],
                                mmcast(hTm[:, (kk * DC + dc) * 128:(kk * DC + dc + 1) * 128]),
                                mmcast(hT[:, dc * T + w0: dc * T + w0 + 512]),
                                start=(dc == 0), stop=(dc == DC - 1),
                            )
                        # masked copy PSUM -> SBUF: sc = S + mask (0 / -3e38)
                        nc.vector.scalar_tensor_tensor(
                            scs[kk][:, w0:w0 + 512], ps[:], 1.0, mskt[:],
                            AluOpType.mult, AluOpType.add,
                        )

                def seg16_into(dst16, ap, W, tagsfx, rr=r):
                    """Top-16 candidates of `ap` ([P, W<=512]) into dst16
                    ([P, 16]): top-8, then ranks 9-16 via match_replace.  The
                    scratch rides the shared bufs=1 [P, 512] "scr" tag
                    (sequential reuses serialize via WAR)."""
                    nc.vector.max(out=dst16[:, 0:8], in_=ap)
                    scr = work.tile([P, W], F32, tag="scr",
                                    bufs=1, name=f"scr_{rr}_{tagsfx}")
                    nc.vector.match_replace(out=scr[:], in_to_replace=dst16[:, 0:8],
                                            in_values=ap, imm_value=NEG_MASK)
                    nc.vector.max(out=dst16[:, 8:16], in_=scr[:])

                def top16_th(sc_ap, W, tagsfx, rr=r):
                    """16th-largest source AP over a [P, W] score range,
                    per-512-segment candidates then a merge pass — a flat
                    match_replace over W=2048 would need an 8KB/partition
                    scratch; segments cap it at 2KB."""
                    nseg = (W + 511) // 512
                    candc = work.tile([P, nseg * 16], F32, tag="candc",
                                      bufs=1, name=f"candc_{rr}_{tagsfx}")
                    for s_ in range(nseg):
                        lo = s_ * 512
                        seg16_into(candc[:, s_ * 16:(s_ + 1) * 16],
                                   sc_ap[:, lo:min(lo + 512, W)],
                                   min(512, W - lo), f"{tagsfx}{s_}")
                    if nseg == 1:
                        return candc[:, 8:16]
                    mg = work.tile([P, 16], F32, tag="mg", bufs=1,
                                   name=f"mg_{rr}_{tagsfx}")
                    seg16_into(mg[:], candc[:], nseg * 16, f"{tagsfx}m")
                    return mg[:, 8:16]

                hb_pre = {}
                _hbalt = [0]

                def hb_copy(jc, h_, rr=r):
                    """fp32 hrows chunk -> rotating bf16 tile (the rounding
                    producer the BIR verifier wants for bf16 matmuls).
                    DVE-heavy 2:1 engine split: the modeled DVE copy is
                    0.39us vs ACT's 0.61us for [128, 512]."""
                    hb = work.tile([P, 512], BF16, tag="hb", bufs=6,
                                   name=f"hb_{rr}_{jc}_{h_}")
                    hop = hrows[:, jc * D + h_ * 512: jc * D + h_ * 512 + 512]
                    _hbalt[0] = (_hbalt[0] + 1) % 3
                    if _hbalt[0] == 0:
                        nc.scalar.activation(hb[:], hop, AF.Copy)
                    else:
                        nc.vector.tensor_copy(hb[:], hop)
                    return hb

                def slot_rest(k, rr=r, th_src_in=None, fast_agg=False):
                    """topk -> mask -> aggregate -> update -> defer prop."""
                    nonlocal pending_prop
                    g = cfg.g[k]
                    W = g * 128
                    sc = scs[k]
                    # ---- top-K threshold ----
                    if th_src_in is not None:
                        th_src = th_src_in
                    elif K <= 8:
                        mx = work.tile([P, 8], F32, tag="mx", name=f"mx_{rr}_{k}")
                        nc.vector.max(out=mx[:], in_=sc[:])
                        th_src = mx[:, K - 1:K]
                    else:
                        t16 = top16_th(sc, W, f"f{k}")
                        th_src = t16[:, K - 9:K - 8]
                    th = work.tile([P, 1], F32, tag="th", name=f"th_{rr}_{k}")
                    nc.vector.tensor_scalar_max(th[:], th_src, NEG_CLAMP)
                    # ---- 0/1 neighbor mask.  MUST NOT ride the Pool queue:
                    # collectives live there, and any instruction queued
                    # behind one waits for its (120us) completion. ----
                    m01 = work.tile([P, W], BF16, tag="m01", bufs=1, name=f"m01_{rr}_{k}")
                    if fast_agg:
                        # split so the first mask-transpose group (and with
                        # it the whole tail aggregation) starts one 512-wide
                        # compare earlier
                        for w_ in range(0, W, 512):
                            nc.vector.tensor_scalar(
                                m01[:, w_:w_ + 512], sc[:, w_:w_ + 512],
                                th[:], None, AluOpType.is_ge)
                    else:
                        nc.vector.tensor_scalar(m01[:], sc[:], th[:], None,
                                                AluOpType.is_ge)
                    # ---- transpose mask chunks + aggregate (interleaved):
                    # msg_raw = M01 @ hrows, accumulated 4 j-chunks at a time
                    # through a small rotating transposed-mask tile ----
                    pss = [psum.tile([P, 512], F32, tag="ps_ag", bufs=2,
                                     name=f"pa_{rr}_{k}_{h_}") for h_ in range(NH)]
                    for jq in range(0, g, 4):
                        ptm = psum.tile([P, 512], BF16, tag="ptb", bufs=2,
                                        name=f"ptm_{rr}_{k}_{jq}")
                        for i4 in range(4):
                            nc.tensor.transpose(
                                ptm[:, i4 * 128:(i4 + 1) * 128],
                                m01[:, (jq + i4) * 128:(jq + i4 + 1) * 128], idb[:])
                        # fast_agg (last round, tail slot only): bf16
                        # aggregation at 1 PE cycle/row instead of fp32's 4.
                        # The mask operand is 0/1 (exact); the h operand is
                        # rounded to bf16 by ACT/DVE chunk copies (the BIR
                        # verifier requires low-precision matmul operands to
                        # come from rounding producers, so no bitcast).
                        # Aggregation error here reaches only the OUTPUT
                        # (~1e-4 rel, no further top-k rounds to flip);
                        # earlier rounds stay fp32 — their msg error would
                        # perturb the next round's scores ~45x and flip
                        # selections.
                        mdt = BF16 if fast_agg else CD
                        mts = work.tile([P, 512], mdt, tag="mt",
                                        bufs=2 if not cfg.affine else 1,
                                        name=f"mt_{rr}_{k}_{jq}")
                        copy_out(mts[:], ptm[:])
                        for h_ in range(NH):
                            for i4 in range(4):
                                jc = jq + i4
                                hop = hrows[:, jc * D + h_ * 512: jc * D + h_ * 512 + 512]
                                if fast_agg:
                                    hb = hb_pre.pop((jc, h_), None)
                                    if hb is None:
                                        hb = hb_copy(jc, h_)
                                    hop = hb[:]
                                nc.tensor.matmul(
                                    pss[h_][:], mmcast(mts[:, i4 * 128:(i4 + 1) * 128]),
                                    mmcast(hop),
                                    start=(jc == 0), stop=(jc == g - 1))
                    # ---- elementwise update ----
                    w2ap = w2t[:, rr * S + k: rr * S + k + 1]
                    EW = 256 if fast_agg else 512
                    for h2 in range(NH * 512 // EW):
                        h_, qo = divmod(h2, 512 // EW)
                        qo *= 256
                        sl = slice(k * D + h_ * 512 + qo, k * D + h_ * 512 + qo + EW)
                        hsl = slice(h_ * 512 + qo, h_ * 512 + qo + EW)
                        t1 = work.tile([P, EW], F32, tag="t1",
                                       bufs=2 if not cfg.affine else 1,
                                       name=f"t1_{rr}_{k}_{h2}")
                        nc.vector.scalar_tensor_tensor(
                            t1[:], pss[h_][:, qo:qo + EW], w2ap, myh[:, sl],
                            AluOpType.mult, AluOpType.add)
                        if cfg.affine:
                            gmt = work.tile([P, 512], F32, tag="gmt", bufs=2,
                                            name=f"gmt_{rr}_{k}_{h_}")
                            nc.scalar.dma_start(gmt[:], i_gm[rr][:, hsl])
                            bbt = work.tile([P, 512], F32, tag="bbt", bufs=2,
                                            name=f"bbt_{rr}_{k}_{h_}")
                            nc.scalar.dma_start(bbt[:], i_bb[rr][:, hsl])
                            nc.vector.tensor_mul(t1[:], t1[:], gmt[:])
                            nc.vector.tensor_add(t1[:], t1[:], bbt[:])
                        gl = work.tile([P, EW], F32, tag="gl",
                                       bufs=2 if not cfg.affine else 1,
                                       name=f"gl_{rr}_{k}_{h2}")
                        if cfg.affine:
                            nc.scalar.activation(gl[:], t1[:], AF.Gelu)
                        else:
                            # u = (psum*w2 + h) * mix  (w2 carries the /mix)
                            nc.scalar.activation(gl[:], t1[:], AF.Gelu,
                                                 scale=sct[:, 4 + rr:5 + rr])
                        if rr < R - 1:
                            nc.vector.tensor_scalar_mul(gl[:], gl[:], ap_1m)
                            # in-place: myh = mom*myh + (1-mom)*gelu
                            nc.vector.scalar_tensor_tensor(
                                myh[:, sl], myh[:, sl], ap_mom, gl[:],
                                AluOpType.mult, AluOpType.add)
                        else:
                            xst = work.tile([P, EW], F32, tag="xst",
                                            bufs=2,
                                            name=f"xst_{rr}_{k}_{h2}")
                            nc.sync.dma_start(xst[:], i_xs[:, sl])
                            # gl <- s*(1-mom)*gelu - s*x
                            nc.vector.scalar_tensor_tensor(
                                gl[:], gl[:], ap_s1m, xst[:],
                                AluOpType.mult, AluOpType.subtract)
                            # t1 <- s*mom*h + gl
                            nc.vector.scalar_tensor_tensor(
                                t1[:], myh[:, sl], ap_sm, gl[:],
                                AluOpType.mult, AluOpType.add)
                            nc.sync.dma_start(o_out[k, :, hsl], t1[:])
                    # ---- propagate h' (transposed into hTm + staged rows) ----
                    if rr < R - 1:
                        def _prop(kk=k):
                            if cfg.bf16:
                                hi_t = work.tile([P, D], CD, tag="hi_t", bufs=1,
                                                 name=f"hi_{rr}_{kk}")
                                nc.scalar.activation(
                                    hi_t[:], myh[:, kk * D:(kk + 1) * D], AF.Copy)
                                src = hi_t[:]
                            else:
                                src = myh[:, kk * D:(kk + 1) * D]
                            # staging DMA FIRST: it gates the phase
                            # collective (the whole chain for slot 0 of
                            # round 0) and needs only the updated myh; the
                            # hTm transposes are for the NEXT round's
                            # scores, ~70us away.
                            p_, q_ = PQ[rr][kk]
                            nc.sync.dma_start(ag_in[rr][p_][q_], src)
                            # fire the phase collective as soon as all its
                            # slots are staged; it only touches DRAM, so it
                            # overlaps the remaining slots' compute.
                            if q_ == len(PHS[rr][p_]) - 1 and p_ < len(PHS[rr]) - 1:
                                nc.gpsimd.collective_compute(
                                    "AllGather", AluOpType.bypass,
                                    replica_groups=cfg.groups,
                                    ins=[ag_in[rr][p_].opt()],
                                    outs=[ag_out[rr][p_].opt()])
                            for half in range(2):
                                pto = psum.tile([P, 512], TD, tag="pt", bufs=2,
                                                name=f"pto_{rr}_{kk}_{half}")
                                for i4 in range(4):
                                    dc = half * 4 + i4
                                    nc.tensor.transpose(
                                        pto[:, i4 * 128:(i4 + 1) * 128],
                                        src[:, dc * 128:(dc + 1) * 128],
                                        idc[:] if cfg.bf16 else idf[:])
                                copy_out(hTm[:, (kk * DC + half * 4) * 128:
                                              (kk * DC + half * 4 + 4) * 128], pto[:])
                        pending_prop = _prop

                def fire_prop():
                    nonlocal pending_prop
                    if pending_prop is not None:
                        with tc.high_priority():
                            pending_prop()
                        pending_prop = None

                # ---- the round's emission schedule.  The PE FIFO is strict,
                # so emission order = execution order: score strips over
                # already-built hT blocks are emitted before rebuilds whose
                # back-DMA may still be waiting on a phase collective, and
                # slot 3's early strips run during the phase-1 wait.
                if r == 0:
                    for k in range(S):
                        score_strips(k, range(0, cfg.g[k] * 128, 512))
                        fire_prop()
                        slot_rest(k)
                        if k == 0:
                            rest_loads()
                        if k < S - 1:
                            # hT/hrows for the NEXT slot
                            bulk_load(k + 1, gate=w2dma)
                else:
                    # scheduler fence at the round boundary: without it the
                    # static scheduler interleaves this round's DVE/ACT ops
                    # before the previous round's tail updates in the per-
                    # engine queues, stalling the staging of the boundary
                    # collectives behind collective-gated work.
                    tc.no_sync_barrier()

                    def fire_backs(at_slot):
                        em = pending_backs[0].pop(at_slot, None)
                        if em is not None:
                            em()

                    fire_backs(0)
                    build_blocks(0, G)
                    score_strips(0, [0])
                    slot_rest(0)
                    score_strips(1, [0])
                    fire_backs(1)
                    build_blocks(G, 2 * G)
                    score_strips(1, [512])
                    fire_prop()  # prop(0): stages the next boundary's phase 0
                    slot_rest(1)
                    score_strips(2, [0, 512])
                    fire_prop()  # prop(1) EARLY: its staging must not queue
                    #              behind the phase-dependent strips below
                    score_strips(3, [0, 512])
                    tc.no_sync_barrier()  # keep the phase-gated rebuild below
                    #                       from being scheduled before the
                    #                       ready work above
                    fire_backs(2)
                    build_blocks(2 * G, 3 * G)
                    score_strips(2, [1024])
                    score_strips(3, [1024])
                    slot_rest(2)
                    fire_prop()  # prop(2) early, same reason
                    if r == R - 1:
                        # Prefix top-16 candidates over the already-computed
                        # [0, 1536) scores, emitted AFTER slot-2's DVE chain
                        # so it doesn't push slot-2's (zero-slack) work into
                        # the tail.  Runs hidden under the last collective.
                        candt = work.tile([P, 64], F32, tag="candc", bufs=1,
                                          name=f"candt_{r}_3")
                        for s_ in range(3):
                            seg16_into(candt[:, s_ * 16:(s_ + 1) * 16],
                                       scs[3][:, s_ * 512:(s_ + 1) * 512],
                                       512, f"p3{s_}")
                    tc.no_sync_barrier()
                    fire_backs(3)
                    if r < R - 1:
                        build_blocks(3 * G, 4 * G)
                        score_strips(3, [1536])
                        slot_rest(3)
                    else:
                        # ---- last-round tail: everything after the final
                        # collective is serial program end, so the stripe-3
                        # work is pipelined per BLOCK (each block's hT
                        # rebuild + 128-wide score segment fires as soon as
                        # its own rank's back-DMA lands) and the top-16
                        # threshold merges the hidden prefix candidates with
                        # candidates from the just-arrived 512-wide suffix
                        # instead of re-scanning all 2048 columns. ----
                        ensure_sc(3)
                        w0 = 1536
                        mskt = work.tile([P, 512], BF16, tag="msk", bufs=2,
                                         name=f"msk_{r}_3_tail")
                        nc.scalar.dma_start(
                            mskt[:], i_msk[:, cfg.OFF[3] + w0:cfg.OFF[3] + w0 + 512])
                        ps = psum.tile([P, 512], F32, tag="ps_sc", bufs=2,
                                       name=f"ps_{r}_3_tail")
                        # builds and strip segments interleaved one build
                        # ahead: each build's psum->SBUF copy costs a DVE
                        # round-trip, which the PREVIOUS block's strip
                        # matmuls hide.
                        def strip_seg(q):
                            for dc in range(DC):
                                nc.tensor.matmul(
                                    ps[:, q * 128:(q + 1) * 128],
                                    mmcast(hTm[:, (3 * DC + dc) * 128:(3 * DC + dc + 1) * 128]),
                                    mmcast(hT[:, dc * T + w0 + q * 128:
                                              dc * T + w0 + (q + 1) * 128]),
                                    start=(dc == 0), stop=(dc == DC - 1),
                                )
                        build_blocks(3 * G, 3 * G + 1)
                        build_blocks(3 * G + 1, 3 * G + 2)
                        strip_seg(0)
                        build_blocks(3 * G + 2, 3 * G + 3)
                        strip_seg(1)
                        build_blocks(3 * G + 3, 3 * G + 4)
                        strip_seg(2)
                        strip_seg(3)
                        nc.vector.scalar_tensor_tensor(
                            scs[3][:, w0:w0 + 512], ps[:], 1.0, mskt[:],
                            AluOpType.mult, AluOpType.add,
                        )
                        for (jc_, h2_) in [(0, 0), (1, 0), (2, 0),
                                           (3, 0), (0, 1), (1, 1)]:
                            hb_pre[(jc_, h2_)] = hb_copy(jc_, h2_)
                        seg16_into(candt[:, 48:64], scs[3][:, w0:w0 + 512],
                                   512, "s3")
                        mgt = work.tile([P, 16], F32, tag="mg", bufs=1,
                                        name=f"mgt_{r}_3")
                        seg16_into(mgt[:], candt[:], 64, "m3")
                        slot_rest(3, th_src_in=mgt[:, 15:16], fast_agg=True)

                fire_prop()  # prop(3)

                # ---- round boundary: last phase collective + back-DMAs ----
                # Back-DMAs are emitted after the slot loop so Tile orders
                # them after this round's readers of hrows.  The fence keeps
                # the scheduler from hoisting them ahead of this round's tail
                # ops on the shared DMA queues (priority inversion: a back-DMA
                # WAR-waits on this round's aggregation, which itself needs a
                # psum copy queued behind that same back-DMA).
                if r < R - 1:
                    tc.no_sync_barrier()
                    nph = len(PHS[r])
                    nc.gpsimd.collective_compute(
                        "AllGather", AluOpType.bypass, replica_groups=cfg.groups,
                        ins=[ag_in[r][nph - 1].opt()], outs=[ag_out[r][nph - 1].opt()])

                    def _backs(p_, eng, rr=r):
                        # one DMA per (phase, q, rank): finer pieces let the
                        # tail's per-block hT rebuilds start as soon as THEIR
                        # rank's rows land instead of waiting out one big
                        # strided transfer on the serial DMA device.
                        nq = len(PHS[rr][p_])
                        srcv = ag_out[rr][p_][:].rearrange(
                            "(rnk q) p d -> q rnk p d", q=nq)
                        for q in range(nq):
                            base = PHS[rr][p_][q] * G
                            for rnk in range(G):
                                for hf in range(2):
                                    eng.dma_start(
                                        hrows[:, (base + rnk) * D + hf * (D // 2):
                                               (base + rnk) * D + (hf + 1) * (D // 2)],
                                        srcv[q, rnk, :, hf * (D // 2):(hf + 1) * (D // 2)])

                    # ALL back-DMAs are deferred into the next round's
                    # schedule (sync queue), each emitted just before the
                    # hT rebuild that consumes it: emitted any earlier they
                    # sit on a DMA queue WAITING on their phase collective,
                    # blocking later traffic on that queue (mask loads,
                    # staging) that the next round needs much sooner.
                    pending_backs[0] = {
                        PHS[r][p_][0]: (lambda rr=r, pp=p_: _backs(pp, nc.sync, rr))
                        for p_ in range(nph)}

    nc.compile()
    return nc


# ------------------------------------------------------------------
# Host side
# ------------------------------------------------------------------

def _sigmoid(v):
    return 1.0 / (1.0 + math.exp(-float(v)))


def prep_inputs(cfg: Cfg, x, gain, bias, log_mix, log_momentum, log_scale):
    """Build the per-core input maps (numpy)."""
    P, D, T, DC, S, G, R = cfg.P, cfg.D, cfg.T, cfg.DC, cfg.S, cfg.G, cfg.R
    cd_np = BF16_NP if cfg.bf16 else np.float32
    x = np.asarray(x, np.float32)
    gain = np.asarray(gain, np.float32)
    bias = np.asarray(bias, np.float32)
    mix = np.array([_sigmoid(v) for v in np.asarray(log_mix, np.float32)], np.float64)
    mom = _sigmoid(log_momentum)
    s = math.log1p(math.exp(float(log_scale))) + 0.01

    scl = np.zeros((P, 8), np.float32)
    scl[:, 0] = mom
    scl[:, 1] = s * (1.0 - mom)
    scl[:, 2] = s * mom
    scl[:, 3] = 1.0 - mom
    scl[:, 4:4 + R] = mix.astype(np.float32)[None, :]
    idc = np.eye(128, dtype=cd_np)

    common = {"i_sc": scl, "i_idc": idc, "i_idb": np.eye(128, dtype=BF16_NP),
              "i_idf": np.eye(128, dtype=np.float32)}
    if cfg.affine:
        common["i_gm"] = np.ascontiguousarray(np.broadcast_to(
            (gain * mix[:, None].astype(np.float32)).astype(np.float32)[:, None, :],
            (R, P, D)))
        common["i_bb"] = np.ascontiguousarray(np.broadcast_to(bias[:, None, :], (R, P, D)))

    in_maps = []
    for c in range(cfg.n_cores):
        b, cc = divmod(c, G)
        blocks = [cc + G * k for k in range(S)]
        h0 = x[b]  # [T, D]
        hT0 = np.ascontiguousarray(
            h0.T.reshape(DC, 128, T).transpose(1, 0, 2)).reshape(128, DC * T).astype(cd_np)
        hr0 = np.ascontiguousarray(
            h0.astype(cd_np).reshape(cfg.NBLK, 128, D).transpose(1, 0, 2)).reshape(128, cfg.NBLK * D)
        hb = h0.reshape(cfg.NBLK, 128, D)[blocks]  # [S,128,D]
        myh0 = np.ascontiguousarray(hb.transpose(1, 0, 2)).reshape(128, S * D)
        hTm0 = np.ascontiguousarray(
            hb.reshape(S, 128, DC, 128).transpose(3, 0, 2, 1)).reshape(128, S * D).astype(cd_np)
        msk = np.zeros((P, cfg.MTOT), BF16_NP)
        for k in range(S):
            W = cfg.g[k] * 128
            rowid = blocks[k] * 128 + np.arange(128)  # [128]
            j = np.arange(W)
            m = np.where(j[None, :] <= rowid[:, None], 0.0, NEG_MASK).astype(BF16_NP)
            msk[:, cfg.OFF[k]:cfg.OFF[k] + W] = m
        w2 = np.zeros((P, R * S), np.float32)
        for r in range(R):
            for k in range(S):
                cnt = np.minimum(blocks[k] * 128 + np.arange(128) + 1, K_SCHEDULE[r])
                w2[:, r * S + k] = ((1.0 - mix[r]) / (mix[r] * cnt)).astype(np.float32)
        in_maps.append(dict(common,
                            i_hT=hT0, i_hr=hr0, i_myh=myh0, i_hTm=hTm0,
                            i_msk=msk, i_xs=(s * myh0.astype(np.float64)).astype(np.float32),
                            i_w2=w2))
    return in_maps


def assemble_output(cfg: Cfg, results, dtype=np.float32):
    """results: list (per core) of {'o_out': [S,128,D]} -> full [B,T,D]."""
    out = np.zeros((cfg.B, cfg.T, cfg.D), dtype)
    for c in range(cfg.n_cores):
        b, cc = divmod(c, cfg.G)
        o = results[c]["o_out"]
        for k in range(cfg.S):
            blk = cc + cfg.G * k
            out[b, blk * 128:(blk + 1) * 128] = o[k]
    return out


_PROGRAM_CACHE = {}


def _get_program(cfg: Cfg):
    key = (cfg.B, cfg.T, cfg.D, cfg.G, cfg.S, cfg.bf16, cfg.affine)
    if key not in _PROGRAM_CACHE:
        _PROGRAM_CACHE[key] = build_program(cfg)
    return _PROGRAM_CACHE[key]


def run(cfg: Cfg, inputs: dict, trace: bool = False):
    nc = _get_program(cfg)
    in_maps = prep_inputs(cfg, **inputs)
    res = bass_utils.run_bass_kernel_spmd(
        nc, in_maps, list(range(cfg.n_cores)), trace=trace)
    out = assemble_output(cfg, res.results)
    return out, res


def kernel(**inputs) -> np.ndarray:
    trivial = (np.all(np.asarray(inputs["gain"]) == 1.0)
               and np.all(np.asarray(inputs["bias"]) == 0.0))
    cfg = Cfg(affine=not trivial)  # B=2, T=2048, D=1024, 8 cores
    out, _ = run(cfg, inputs)
    return out.astype(np.float32)

